# revision 30
# baseline (speedup 1.0000x reference)
"""GQA attention prefill kernel for Trainium2 (Bass/Tile), 8-way tensor
parallel over heads.

Problem (hardcoded): B=1, S=2048, HID=4096, NH=32, KVH=8, D=128, causal
prefill with per-head RMSNorm on q/k and RoPE, positions = arange(S).

Sharding: core c owns kv-head c and q-heads 4c..4c+3. wq/wo sharded on the
head dim, wk/wv on the kv-head dim; x, rope tables replicated. Each core
computes its 4 heads' contribution through wo; the host sums the 8 partial
outputs (partials shipped as bf16, summed in fp32).

All matmul operands are bf16 (PE runs 1 cycle/row and FWL halves weight
loads; fp32r measured ~2.2 cycles/row on HW). PSUM accumulation stays fp32.
The rotate-half sign is folded into the sin table host-side; the q/k norm
weights are applied on-device as a per-partition scalar in the fused
(pre * w) * rsqrt multiply.

Weights/activations are shipped in [partition, tile, free] 3-D layouts so
every SBUF load is one strided DMA descriptor (the Sync sequencer pays
~600 ns per dma_start; v1 of this kernel lost ~250 us to descriptor issue).
Big transfers are split across a few descriptors so multiple DMA engines
run in parallel (one queue sustains only ~24 GB/s).

Structure (per chunk of 512 q positions), software-pipelined:
  proj(c+1) -> attention(c) with outproj(c-1) matmuls interleaved as PE
  filler while the scalar engine grinds exp.
Projections are head-serial (one PSUM bank at a time, x chunk resident in
SBUF) so only 2 proj banks are ever live; PSUM budget is
2 (proj+outproj) + 3 (scores) + 2 (softmax denom) + 1 (att out) = 8 banks.

RMS-norm cross-partition sums run on GpSimd (partition_all_reduce) instead
of PE ones-matmuls; reciprocals use the fast DVE approximation. The causal
diagonal k-tiles compute only the valid q columns (free-dim trim).
"""

import numpy as np

import concourse.bass as bass
import concourse.mybir as mybir
import concourse.tile as tile
from concourse import bacc
from concourse import bass_isa
from concourse.masks import make_identity

P = 128
S = 2048
HID = 4096
D = 128
G = 4            # q heads per core
NHT = HID // P   # 32 h-tiles (contraction)
SC = 512         # seq chunk
NSC = S // SC    # 4
NKT = S // P     # 16 k-tiles
EPS = 1e-6
N_CORES = 8

F32 = mybir.dt.float32
BF16 = mybir.dt.bfloat16
MULT = mybir.AluOpType.mult


def build_program():
    nc = bacc.Bacc("TRN2", target_bir_lowering=False, debug=False)

    xT = nc.dram_tensor("xT", [P, NHT, S], BF16, kind="ExternalInput").ap()
    wqT = nc.dram_tensor("wqT", [P, NHT, G * P], BF16,
                         kind="ExternalInput").ap()
    wkT = nc.dram_tensor("wkT", [P, NHT, P], BF16, kind="ExternalInput").ap()
    wvT = nc.dram_tensor("wvT", [P, NHT, P], BF16, kind="ExternalInput").ap()
    woT = nc.dram_tensor("woT", [P, G, HID], BF16, kind="ExternalInput").ap()
    cost = nc.dram_tensor("cost", [D, S], BF16, kind="ExternalInput").ap()
    sint = nc.dram_tensor("sint", [D, S], BF16, kind="ExternalInput").ap()
    qnw = nc.dram_tensor("qnw", [D, 1], F32, kind="ExternalInput").ap()
    knw = nc.dram_tensor("knw", [D, 1], F32, kind="ExternalInput").ap()
    y = nc.dram_tensor("y", [S, HID], BF16, kind="ExternalOutput").ap()

    Sqrt = mybir.ActivationFunctionType.Sqrt
    Exp = mybir.ActivationFunctionType.Exp

    with tile.TileContext(nc) as tc:
        with (
            tc.tile_pool(name="const", bufs=1) as const,
            tc.tile_pool(name="wres", bufs=1) as wres,
            tc.tile_pool(name="xp", bufs=2) as xp,
            tc.tile_pool(name="qrp", bufs=2) as qrp,
            tc.tile_pool(name="wog", bufs=3) as wogp,
            tc.tile_pool(name="scr", bufs=3) as scr,
            tc.tile_pool(name="ptp", bufs=3) as ptp,
            tc.tile_pool(name="otp", bufs=8) as otp,
            tc.tile_pool(name="rcpp", bufs=2) as rcpp,
            tc.tile_pool(name="ysp", bufs=6) as ysp,
            # PSUM: exactly 8 banks total.
            tc.tile_pool(name="psA", bufs=2, space="PSUM") as psA,  # proj+outproj
            tc.tile_pool(name="psB", bufs=3, space="PSUM") as psB,  # scores+transp
            tc.tile_pool(name="psC", bufs=2, space="PSUM") as psC,  # softmax denom
            tc.tile_pool(name="psD", bufs=1, space="PSUM") as psD,  # att out
        ):
            # ---- resident tensors (batched loads, interleaved so the
            # first projection stream unblocks early) ----
            wq_sb = wres.tile([P, NHT, G * P], BF16)
            wk_sb = wres.tile([P, NHT, P], BF16)
            wv_sb = wres.tile([P, NHT, P], BF16)
            x_tiles = [xp.tile([P, NHT, SC], BF16, tag="xall", name=f"x{c}")
                       for c in range(NSC)]
            # one DMA engine moves only ~24 GB/s, so keep pieces <=128KB and
            # let them land on parallel engines. Partition-split the very
            # first x/wv pieces so the first matmul (V stream) starts early.
            for pp in (slice(0, 64), slice(64, 128)):
                nc.sync.dma_start(x_tiles[0][pp, 0:1, :], xT[pp, 0:1, 0:SC])
                nc.sync.dma_start(wv_sb[pp, 0:2, :], wvT[pp, 0:2, :])
            nc.sync.dma_start(x_tiles[0][:, 1:2, :], xT[:, 1:2, 0:SC])
            for i in range(1, 16):
                h2 = slice(i * 2, (i + 1) * 2)
                nc.sync.dma_start(x_tiles[0][:, h2, :], xT[:, h2, 0:SC])
                if i >= 1:
                    nc.sync.dma_start(wv_sb[:, h2, :], wvT[:, h2, :])
            for i in range(8):
                h4 = slice(i * 4, (i + 1) * 4)
                nc.sync.dma_start(wk_sb[:, h4, :], wkT[:, h4, :])
                nc.sync.dma_start(wq_sb[:, h4, :], wqT[:, h4, :])
            cs = wres.tile([P, S], BF16, name="cs")
            sn = wres.tile([P, S], BF16, name="sn")
            for i in range(4):
                sl = slice(i * S // 4, (i + 1) * S // 4)
                nc.sync.dma_start(cs[:, sl], cost[:, sl])
                nc.sync.dma_start(sn[:, sl], sint[:, sl])
            qn = const.tile([P, 1], F32, name="qn")
            kn = const.tile([P, 1], F32, name="kn")
            nc.sync.dma_start(qn, qnw)
            nc.sync.dma_start(kn, knw)

            # ---- constants ----
            identity = const.tile([P, P], BF16)
            make_identity(nc, identity)
            f32tmp = const.tile([P, SC], F32)
            ones_bf = const.tile([P, P], BF16)
            nc.gpsimd.memset(f32tmp, 1.0)
            nc.vector.tensor_copy(ones_bf, f32tmp[:, 0:P])
            # causal masks for the 4 diagonal k-tiles of a q chunk:
            # keep (1.0) where q_local >= 128*j + k_local
            masks = []
            for j in range(4):
                mk = const.tile([P, SC], BF16, name=f"mask{j}")
                nc.gpsimd.memset(f32tmp, 1.0)
                nc.gpsimd.affine_select(
                    f32tmp, f32tmp, pattern=[[1, SC]],
                    compare_op=mybir.AluOpType.is_ge,
                    fill=0.0, base=-P * j, channel_multiplier=-1,
                )
                nc.vector.tensor_copy(mk, f32tmp)
                masks.append(mk)

            bias_keps = const.tile([P, 1], F32)
            nc.gpsimd.memset(bias_keps, float(P) * EPS)
            bias_qeps = const.tile([P, 1], F32)
            nc.gpsimd.memset(bias_qeps, EPS)

            KR = wres.tile([P, S], BF16, name="KR")       # roped+scaled K [d, s]
            Vs = wres.tile([P, NKT, P], BF16, name="Vs")  # V [s-in-tile, kt, d]

            # ot tiles per (chunk, head) — kept alive until outproj(c) done
            ots = {}

            def emit_proj(c):
                """Projections + rope/norm for chunk c. Head-serial: one
                PSUM bank per stream. The rope chain is pipelined one
                stream deep: stream s+1's PSUM evac is emitted before
                stream s's sqrt/rope tail, so the ACT queue never blocks
                on the GpSimd partition-reduce."""
                q0 = c * SC
                xall = x_tiles[c]
                qr_t = qrp.tile([P, G, SC], BF16, tag="qr", name=f"qr{c}")

                def evac(pj):
                    """PSUM evac + square-sum launch + rotate-half DMAs.
                    The rot DMAs ride the Activation HWDGE queue so they
                    never wait behind bulk x/w transfers on Sync's."""
                    raw = scr.tile([P, SC], BF16, tag="raw")
                    nc.scalar.copy(raw, pj)
                    rot = scr.tile([P, SC], BF16, tag="rot")
                    nc.sync.dma_start(rot[0:64], raw[64:128])
                    nc.sync.dma_start(rot[64:128], raw[0:64])
                    sqv = scr.tile([P, SC], BF16, tag="sqv")
                    nc.vector.tensor_mul(sqv, raw, raw)
                    ssq = scr.tile([P, SC], F32, tag="ssq")
                    nc.gpsimd.partition_all_reduce(
                        ssq, sqv, P, bass_isa.ReduceOp.add)
                    return raw, rot, ssq

                def finish(st, nw, bias, scale, out_ap):
                    raw, rot, ssq = st
                    nc.scalar.activation(ssq, ssq, Sqrt, bias=bias, scale=scale)
                    rkf = scr.tile([P, SC], F32, tag="rkf")
                    nc.vector.reciprocal_approx_fast(rkf, ssq)
                    t1 = scr.tile([P, SC], BF16, tag="t1")
                    nc.vector.tensor_mul(t1, rot, sn[:, q0:q0 + SC])
                    # in-place: raw = raw*cos (rot DMA read already consumed
                    # raw; WAR tracked by the framework)
                    nc.vector.tensor_mul(raw, raw, cs[:, q0:q0 + SC])
                    nc.vector.tensor_add(raw, raw, t1)
                    # out = (raw * norm_w[P,1]) * rsqrt_factor, one DVE op
                    nc.vector.scalar_tensor_tensor(
                        out_ap, raw, nw, rkf, MULT, MULT)

                # v first: its PE transposes share the psB ring with the
                # next attention phase's score tiles, so they must happen
                # early, not at the phase boundary
                pj = psA.tile([P, SC], F32, tag="a", name=f"vp{c}")
                for ht in range(NHT):
                    nc.tensor.matmul(pj, wv_sb[:, ht, :], xall[:, ht, :],
                                     start=(ht == 0), stop=(ht == NHT - 1))
                vtmp = scr.tile([P, SC], BF16, tag="vtmp")
                # ACT, not DVE: the DVE queue backs up with rope chains at
                # the end of the proj phase and would delay the transposes
                nc.scalar.copy(vtmp, pj)
                for j in range(SC // P):
                    tp = psB.tile([P, P], BF16, tag="b", name=f"tp{c}_{j}")
                    nc.tensor.transpose(tp, vtmp[:, j * P:(j + 1) * P], identity)
                    nc.vector.tensor_copy(Vs[:, c * 4 + j, :], tp)
                # k, then the 4 q heads
                pending = None
                for h in range(-1, G):
                    pj = psA.tile([P, SC], F32, tag="a", name=f"qp{c}_{h}")
                    for ht in range(NHT):
                        if h >= 0:
                            lhs = wq_sb[:, ht, h * P:(h + 1) * P]
                        else:
                            lhs = wk_sb[:, ht, :]
                        nc.tensor.matmul(pj, lhs, xall[:, ht, :],
                                         start=(ht == 0), stop=(ht == NHT - 1))
                    st = evac(pj)
                    if pending is not None:
                        pending()
                    if h >= 0:
                        pending = (lambda st=st, h=h: finish(
                            st, qn, bias_qeps, 1.0 / P, qr_t[:, h, :]))
                    else:
                        pending = (lambda st=st: finish(
                            st, kn, bias_keps, 1.0, KR[:, q0:q0 + SC]))
                    del st
                pending()
                # prefetch next chunk's x via the Activation HWDGE queue:
                # on Sync it would either sit ahead of the latency-critical
                # rot DMAs or behind their semaphore-gated issues
                if c + 1 < NSC:
                    for i in range(16):
                        h2 = slice(i * 2, (i + 1) * 2)
                        nc.scalar.dma_start(x_tiles[c + 1][:, h2, :],
                                            xT[:, h2, q0 + SC:q0 + 2 * SC])
                return qr_t

            def outproj_ops(c):
                """Generator of closures: output projection for chunk c,
                in PE-sized steps (one matmul per step). Used as filler
                between attention matmuls of chunk c+1."""
                q0 = c * SC
                wogs = {}

                def load_wog(ng):
                    wog = wogp.tile([P, G, SC], BF16, tag="wog")
                    for h in range(G):
                        nc.sync.dma_start(wog[:, h, :],
                                          woT[:, h, ng * SC:(ng + 1) * SC])
                    wogs[ng] = wog

                # first two groups' weights prefetched by the priming
                # next()-calls issued before proj(c+2) is emitted
                yield lambda: load_wog(0)
                yield lambda: load_wog(1)
                for ng in range(HID // SC):
                    wog = wogs[ng]
                    if ng + 2 < HID // SC:
                        yield lambda ng=ng: load_wog(ng + 2)
                    for stl in range(SC // P):
                        yp = psA.tile([P, SC], F32, tag="a",
                                      name=f"yp{c}_{ng}_{stl}")
                        for h in range(G):
                            yield lambda yp=yp, h=h, stl=stl, wog=wog, c=c: \
                                nc.tensor.matmul(
                                    yp, ots[(c, h)][:, stl * P:(stl + 1) * P],
                                    wog[:, h, :],
                                    start=(h == 0), stop=(h == G - 1))

                        def evac(yp=yp, stl=stl, ng=ng, q0=q0):
                            ys = ysp.tile([P, SC], BF16, tag="ys")
                            # vector only: a copy landing on ACT would evict
                            # the Exp table mid-attention (1.3us reload)
                            nc.vector.tensor_copy(ys, yp)
                            # store via the GpSimd SWDGE queue: keeps 128
                            # store issues off the Sync queue, and per-stl
                            # pieces land on parallel DMA engines
                            nc.gpsimd.dma_start(
                                y[q0 + stl * P:q0 + (stl + 1) * P,
                                  ng * SC:(ng + 1) * SC], ys)
                        yield evac

            def emit_attn(c, qr_t, filler):
                """Attention for chunk c; `filler` ops (outproj of c-1)
                are interleaved to keep PE busy while ACT runs exp."""
                def take(n):
                    for _ in range(n):
                        op = next(filler, None)
                        if op is None:
                            return
                        op()

                # boundary filler: outproj matmuls keep the PE fed while
                # the DVE drains the proj phase's rope-chain backlog
                take(21)
                nkt = (c + 1) * 4
                for h in range(G):
                    avp = psD.tile([P, SC], F32, tag="d", name=f"av{c}_{h}")
                    dnp = psC.tile([P, SC], F32, tag="c", name=f"dn{c}_{h}")
                    for kt in range(nkt):
                        j = kt - c * 4
                        off = P * j if j >= 0 else 0
                        ptps = psB.tile([P, SC], F32, tag="b",
                                        name=f"pt{c}_{h}_{kt}")
                        nc.tensor.matmul(
                            ptps[:, off:], KR[:, kt * P:(kt + 1) * P],
                            qr_t[:, h, off:], start=True, stop=True)
                        pt = ptp.tile([P, SC], BF16, tag="pt")
                        nc.scalar.activation(pt[:, off:], ptps[:, off:], Exp,
                                             bias=0.0, scale=1.0)
                        if j >= 0:
                            nc.vector.tensor_mul(pt[:, off:], pt[:, off:],
                                                 masks[j][:, off:])
                        nc.tensor.matmul(dnp[:, off:], ones_bf, pt[:, off:],
                                         start=(kt == 0), stop=(kt == nkt - 1))
                        nc.tensor.matmul(avp[:, off:], Vs[:, kt, :],
                                         pt[:, off:],
                                         start=(kt == 0), stop=(kt == nkt - 1))
                        take(2)
                    rcp = rcpp.tile([P, SC], F32, tag="rcp")
                    nc.vector.reciprocal_approx_fast(rcp, dnp)
                    ot = otp.tile([P, SC], BF16, tag="ot", name=f"ot{c}_{h}")
                    nc.vector.tensor_mul(ot, avp, rcp)
                    ots[(c, h)] = ot
                    take(7)

            # ---- pipelined main loop ----
            qr_next = emit_proj(0)
            empty = iter(())
            for c in range(NSC):
                qr_cur = qr_next
                filler = outproj_ops(c - 1) if c >= 1 else empty
                if c >= 1:
                    next(filler)()  # prime wog(0) DMA before proj(c+1)
                    next(filler)()  # prime wog(1)
                if c + 1 < NSC:
                    qr_next = emit_proj(c + 1)
                emit_attn(c, qr_cur, filler)
                for op in filler:  # flush leftovers
                    op()
            for op in outproj_ops(NSC - 1):
                op()

    nc.finalize()
    return nc


def shard_inputs(x, wq, wk, wv, wo, q_norm_w, k_norm_w, cos_table, sin_table,
                 positions, **_ignored):
    """Host-side sharding: returns the list of 8 per-core input maps."""
    import ml_dtypes
    bf16 = ml_dtypes.bfloat16

    x = np.asarray(x, np.float32)
    pos = np.asarray(positions).astype(np.int64)
    cos_sel = np.asarray(cos_table, np.float32)[pos]   # [S, D]
    sin_sel = np.asarray(sin_table, np.float32)[pos]
    qw = np.ascontiguousarray(
        np.asarray(q_norm_w, np.float32).reshape(D, 1))
    kw = np.ascontiguousarray(
        np.asarray(k_norm_w, np.float32).reshape(D, 1))
    # fold rotate-half's minus sign into sin rows 0..63:
    # rope(z) = z*cos + [-z2; z1]*sin = z*cos + [z2; z1]*sin_eff
    sign = np.ones((1, D), np.float32)
    sign[0, :D // 2] = -1.0
    cost = np.ascontiguousarray(cos_sel.T).astype(bf16)            # [D, S]
    sint = np.ascontiguousarray((sin_sel * sign).T).astype(bf16)
    # x as [p, ht, s]: x[s, ht*128+p]
    xT3 = np.ascontiguousarray(
        x.reshape(S, NHT, P).transpose(2, 1, 0)).astype(bf16)
    wq = np.asarray(wq, np.float32)
    wk = np.asarray(wk, np.float32)
    wv = np.asarray(wv, np.float32)
    wo = np.asarray(wo, np.float32)

    in_maps = []
    for c in range(N_CORES):
        # weight shards, [p, ht, m] with p the contraction partition
        wq_s = wq[c * G * P:(c + 1) * G * P, :].T     # [HID, 512]
        wk_s = wk[c * P:(c + 1) * P, :].T             # [HID, 128]
        wv_s = wv[c * P:(c + 1) * P, :].T
        wo_s = wo[:, c * G * P:(c + 1) * G * P].T     # [512, HID]
        m = {
            "xT": xT3,
            "wqT": np.ascontiguousarray(
                wq_s.reshape(NHT, P, G * P).transpose(1, 0, 2)).astype(bf16),
            "wkT": np.ascontiguousarray(
                wk_s.reshape(NHT, P, P).transpose(1, 0, 2)).astype(bf16),
            "wvT": np.ascontiguousarray(
                wv_s.reshape(NHT, P, P).transpose(1, 0, 2)).astype(bf16),
            "woT": np.ascontiguousarray(
                wo_s.reshape(G, P, HID).transpose(1, 0, 2)).astype(bf16),
            "cost": cost, "sint": sint, "qnw": qw, "knw": kw,
        }
        in_maps.append(m)
    return in_maps


_NC = None


def _get_nc():
    global _NC
    if _NC is None:
        _NC = build_program()
    return _NC


def run_on_device(in_maps, trace=False):
    from concourse.bass_utils import run_bass_kernel_spmd
    nc = _get_nc()
    return run_bass_kernel_spmd(nc, in_maps, list(range(N_CORES)), trace=trace)


def kernel(**inputs):
    in_maps = shard_inputs(**inputs)
    res = run_on_device(in_maps).results
    y = np.zeros((S, HID), np.float32)
    for c in range(N_CORES):
        y += np.asarray(res[c]["y"], np.float32)
    return y.reshape(1, S, HID)


# revision 32
# speedup vs baseline: 1.0011x; 1.0011x over previous
"""GQA attention prefill kernel for Trainium2 (Bass/Tile), 8-way tensor
parallel over heads.

Problem (hardcoded): B=1, S=2048, HID=4096, NH=32, KVH=8, D=128, causal
prefill with per-head RMSNorm on q/k and RoPE, positions = arange(S).

Sharding: core c owns kv-head c and q-heads 4c..4c+3. wq/wo sharded on the
head dim, wk/wv on the kv-head dim; x, rope tables replicated. Each core
computes its 4 heads' contribution through wo; the host sums the 8 partial
outputs (partials shipped as bf16, summed in fp32).

All matmul operands are bf16 (PE runs 1 cycle/row and FWL halves weight
loads; fp32r measured ~2.2 cycles/row on HW). PSUM accumulation stays fp32.
The rotate-half sign is folded into the sin table host-side; the q/k norm
weights are applied on-device as a per-partition scalar in the fused
(pre * w) * rsqrt multiply.

Weights/activations are shipped in [partition, tile, free] 3-D layouts so
every SBUF load is one strided DMA descriptor (the Sync sequencer pays
~600 ns per dma_start; v1 of this kernel lost ~250 us to descriptor issue).
Big transfers are split across a few descriptors so multiple DMA engines
run in parallel (one queue sustains only ~24 GB/s).

Structure (per chunk of 512 q positions), software-pipelined:
  proj(c+1) -> attention(c) with outproj(c-1) matmuls interleaved as PE
  filler while the scalar engine grinds exp.
Projections are head-serial (one PSUM bank at a time, x chunk resident in
SBUF) so only 2 proj banks are ever live; PSUM budget is
2 (proj+outproj) + 3 (scores) + 2 (softmax denom) + 1 (att out) = 8 banks.

RMS-norm cross-partition sums run on GpSimd (partition_all_reduce) instead
of PE ones-matmuls; reciprocals use the fast DVE approximation. The causal
diagonal k-tiles compute only the valid q columns (free-dim trim).
"""

import numpy as np

import concourse.bass as bass
import concourse.mybir as mybir
import concourse.tile as tile
from concourse import bacc
from concourse import bass_isa
from concourse.masks import make_identity

P = 128
S = 2048
HID = 4096
D = 128
G = 4            # q heads per core
NHT = HID // P   # 32 h-tiles (contraction)
SC = 512         # seq chunk
NSC = S // SC    # 4
NKT = S // P     # 16 k-tiles
EPS = 1e-6
N_CORES = 8

F32 = mybir.dt.float32
BF16 = mybir.dt.bfloat16
MULT = mybir.AluOpType.mult


def build_program():
    nc = bacc.Bacc("TRN2", target_bir_lowering=False, debug=False)

    xT = nc.dram_tensor("xT", [P, NHT, S], BF16, kind="ExternalInput").ap()
    wqT = nc.dram_tensor("wqT", [P, NHT, G * P], BF16,
                         kind="ExternalInput").ap()
    wkT = nc.dram_tensor("wkT", [P, NHT, P], BF16, kind="ExternalInput").ap()
    wvT = nc.dram_tensor("wvT", [P, NHT, P], BF16, kind="ExternalInput").ap()
    woT = nc.dram_tensor("woT", [P, G, HID], BF16, kind="ExternalInput").ap()
    cost = nc.dram_tensor("cost", [D, S], BF16, kind="ExternalInput").ap()
    sint = nc.dram_tensor("sint", [D, S], BF16, kind="ExternalInput").ap()
    qnw = nc.dram_tensor("qnw", [D, 1], F32, kind="ExternalInput").ap()
    knw = nc.dram_tensor("knw", [D, 1], F32, kind="ExternalInput").ap()
    y = nc.dram_tensor("y", [S, HID], BF16, kind="ExternalOutput").ap()

    Sqrt = mybir.ActivationFunctionType.Sqrt
    Exp = mybir.ActivationFunctionType.Exp

    with tile.TileContext(nc) as tc:
        with (
            tc.tile_pool(name="const", bufs=1) as const,
            tc.tile_pool(name="wres", bufs=1) as wres,
            tc.tile_pool(name="xp", bufs=2) as xp,
            tc.tile_pool(name="qrp", bufs=2) as qrp,
            tc.tile_pool(name="wog", bufs=3) as wogp,
            tc.tile_pool(name="scr", bufs=3) as scr,
            tc.tile_pool(name="ptp", bufs=3) as ptp,
            tc.tile_pool(name="otp", bufs=8) as otp,
            tc.tile_pool(name="rcpp", bufs=2) as rcpp,
            tc.tile_pool(name="ysp", bufs=6) as ysp,
            # PSUM: exactly 8 banks total.
            tc.tile_pool(name="psA", bufs=2, space="PSUM") as psA,  # proj+outproj
            tc.tile_pool(name="psB", bufs=3, space="PSUM") as psB,  # scores+transp
            tc.tile_pool(name="psC", bufs=2, space="PSUM") as psC,  # softmax denom
            tc.tile_pool(name="psD", bufs=1, space="PSUM") as psD,  # att out
        ):
            # ---- resident tensors (batched loads, interleaved so the
            # first projection stream unblocks early) ----
            wq_sb = wres.tile([P, NHT, G * P], BF16)
            wk_sb = wres.tile([P, NHT, P], BF16)
            wv_sb = wres.tile([P, NHT, P], BF16)
            x_tiles = [xp.tile([P, NHT, SC], BF16, tag="xall", name=f"x{c}")
                       for c in range(NSC)]
            # one DMA engine moves only ~24 GB/s, so keep pieces <=128KB and
            # let them land on parallel engines. Partition-split the very
            # first x/wv pieces so the first matmul (V stream) starts early.
            for pp in (slice(0, 64), slice(64, 128)):
                nc.sync.dma_start(x_tiles[0][pp, 0:1, :], xT[pp, 0:1, 0:SC])
                nc.sync.dma_start(wv_sb[pp, 0:2, :], wvT[pp, 0:2, :])
            nc.sync.dma_start(x_tiles[0][:, 1:2, :], xT[:, 1:2, 0:SC])
            for i in range(1, 16):
                h2 = slice(i * 2, (i + 1) * 2)
                nc.sync.dma_start(x_tiles[0][:, h2, :], xT[:, h2, 0:SC])
                if i >= 1:
                    nc.sync.dma_start(wv_sb[:, h2, :], wvT[:, h2, :])
            for i in range(8):
                h4 = slice(i * 4, (i + 1) * 4)
                nc.sync.dma_start(wk_sb[:, h4, :], wkT[:, h4, :])
                nc.sync.dma_start(wq_sb[:, h4, :], wqT[:, h4, :])
            cs = wres.tile([P, S], BF16, name="cs")
            sn = wres.tile([P, S], BF16, name="sn")
            for i in range(4):
                sl = slice(i * S // 4, (i + 1) * S // 4)
                nc.sync.dma_start(cs[:, sl], cost[:, sl])
                nc.sync.dma_start(sn[:, sl], sint[:, sl])
            qn = const.tile([P, 1], F32, name="qn")
            kn = const.tile([P, 1], F32, name="kn")
            nc.sync.dma_start(qn, qnw)
            nc.sync.dma_start(kn, knw)

            # ---- constants ----
            identity = const.tile([P, P], BF16)
            make_identity(nc, identity)
            f32tmp = const.tile([P, SC], F32)
            ones_bf = const.tile([P, P], BF16)
            nc.gpsimd.memset(f32tmp, 1.0)
            nc.vector.tensor_copy(ones_bf, f32tmp[:, 0:P])
            # causal masks for the 4 diagonal k-tiles of a q chunk:
            # keep (1.0) where q_local >= 128*j + k_local
            masks = []
            for j in range(4):
                mk = const.tile([P, SC], BF16, name=f"mask{j}")
                nc.gpsimd.memset(f32tmp, 1.0)
                nc.gpsimd.affine_select(
                    f32tmp, f32tmp, pattern=[[1, SC]],
                    compare_op=mybir.AluOpType.is_ge,
                    fill=0.0, base=-P * j, channel_multiplier=-1,
                )
                nc.vector.tensor_copy(mk, f32tmp)
                masks.append(mk)

            bias_keps = const.tile([P, 1], F32)
            nc.gpsimd.memset(bias_keps, float(P) * EPS)
            bias_qeps = const.tile([P, 1], F32)
            nc.gpsimd.memset(bias_qeps, EPS)

            KR = wres.tile([P, S], BF16, name="KR")       # roped+scaled K [d, s]
            Vs = wres.tile([P, NKT, P], BF16, name="Vs")  # V [s-in-tile, kt, d]

            # ot tiles per (chunk, head) — kept alive until outproj(c) done
            ots = {}

            def emit_proj(c):
                """Projections + rope/norm for chunk c. Head-serial: one
                PSUM bank per stream. The rope chain is pipelined one
                stream deep: stream s+1's PSUM evac is emitted before
                stream s's sqrt/rope tail, so the ACT queue never blocks
                on the GpSimd partition-reduce."""
                q0 = c * SC
                xall = x_tiles[c]
                qr_t = qrp.tile([P, G, SC], BF16, tag="qr", name=f"qr{c}")
                # prefetch next chunk's x via the GpSimd SWDGE queue, ahead
                # of this chunk's partition-reduces: Sync would order it
                # against the latency-critical rot DMAs, Activation against
                # the attention exps
                if c + 1 < NSC:
                    for i in range(16):
                        h2 = slice(i * 2, (i + 1) * 2)
                        nc.gpsimd.dma_start(x_tiles[c + 1][:, h2, :],
                                            xT[:, h2, q0 + SC:q0 + 2 * SC])

                def evac(pj):
                    """PSUM evac + square-sum launch + rotate-half DMAs.
                    The rot DMAs ride the Activation HWDGE queue so they
                    never wait behind bulk x/w transfers on Sync's."""
                    raw = scr.tile([P, SC], BF16, tag="raw")
                    nc.scalar.copy(raw, pj)
                    rot = scr.tile([P, SC], BF16, tag="rot")
                    nc.sync.dma_start(rot[0:64], raw[64:128])
                    nc.sync.dma_start(rot[64:128], raw[0:64])
                    sqv = scr.tile([P, SC], BF16, tag="sqv")
                    nc.vector.tensor_mul(sqv, raw, raw)
                    ssq = scr.tile([P, SC], F32, tag="ssq")
                    nc.gpsimd.partition_all_reduce(
                        ssq, sqv, P, bass_isa.ReduceOp.add)
                    return raw, rot, ssq

                def finish(st, nw, bias, scale, out_ap):
                    raw, rot, ssq = st
                    nc.scalar.activation(ssq, ssq, Sqrt, bias=bias, scale=scale)
                    rkf = scr.tile([P, SC], F32, tag="rkf")
                    nc.vector.reciprocal_approx_fast(rkf, ssq)
                    t1 = scr.tile([P, SC], BF16, tag="t1")
                    nc.vector.tensor_mul(t1, rot, sn[:, q0:q0 + SC])
                    # in-place: raw = raw*cos (rot DMA read already consumed
                    # raw; WAR tracked by the framework)
                    nc.vector.tensor_mul(raw, raw, cs[:, q0:q0 + SC])
                    nc.vector.tensor_add(raw, raw, t1)
                    # out = (raw * norm_w[P,1]) * rsqrt_factor, one DVE op
                    nc.vector.scalar_tensor_tensor(
                        out_ap, raw, nw, rkf, MULT, MULT)

                # v first: its PE transposes share the psB ring with the
                # next attention phase's score tiles, so they must happen
                # early, not at the phase boundary
                pj = psA.tile([P, SC], F32, tag="a", name=f"vp{c}")
                for ht in range(NHT):
                    nc.tensor.matmul(pj, wv_sb[:, ht, :], xall[:, ht, :],
                                     start=(ht == 0), stop=(ht == NHT - 1))
                vtmp = scr.tile([P, SC], BF16, tag="vtmp")
                # ACT, not DVE: the DVE queue backs up with rope chains at
                # the end of the proj phase and would delay the transposes
                nc.scalar.copy(vtmp, pj)
                for j in range(SC // P):
                    tp = psB.tile([P, P], BF16, tag="b", name=f"tp{c}_{j}")
                    nc.tensor.transpose(tp, vtmp[:, j * P:(j + 1) * P], identity)
                    nc.vector.tensor_copy(Vs[:, c * 4 + j, :], tp)
                # k, then the 4 q heads
                pending = None
                for h in range(-1, G):
                    pj = psA.tile([P, SC], F32, tag="a", name=f"qp{c}_{h}")
                    for ht in range(NHT):
                        if h >= 0:
                            lhs = wq_sb[:, ht, h * P:(h + 1) * P]
                        else:
                            lhs = wk_sb[:, ht, :]
                        nc.tensor.matmul(pj, lhs, xall[:, ht, :],
                                         start=(ht == 0), stop=(ht == NHT - 1))
                    st = evac(pj)
                    if pending is not None:
                        pending()
                    if h >= 0:
                        pending = (lambda st=st, h=h: finish(
                            st, qn, bias_qeps, 1.0 / P, qr_t[:, h, :]))
                    else:
                        pending = (lambda st=st: finish(
                            st, kn, bias_keps, 1.0, KR[:, q0:q0 + SC]))
                    del st
                pending()
                return qr_t

            def outproj_ops(c):
                """Generator of closures: output projection for chunk c,
                in PE-sized steps (one matmul per step). Used as filler
                between attention matmuls of chunk c+1."""
                q0 = c * SC
                wogs = {}

                def load_wog(ng):
                    wog = wogp.tile([P, G, SC], BF16, tag="wog")
                    for h in range(G):
                        nc.sync.dma_start(wog[:, h, :],
                                          woT[:, h, ng * SC:(ng + 1) * SC])
                    wogs[ng] = wog

                # first two groups' weights prefetched by the priming
                # next()-calls issued before proj(c+2) is emitted
                yield lambda: load_wog(0)
                yield lambda: load_wog(1)
                for ng in range(HID // SC):
                    wog = wogs[ng]
                    if ng + 2 < HID // SC:
                        yield lambda ng=ng: load_wog(ng + 2)
                    for stl in range(SC // P):
                        yp = psA.tile([P, SC], F32, tag="a",
                                      name=f"yp{c}_{ng}_{stl}")
                        for h in range(G):
                            yield lambda yp=yp, h=h, stl=stl, wog=wog, c=c: \
                                nc.tensor.matmul(
                                    yp, ots[(c, h)][:, stl * P:(stl + 1) * P],
                                    wog[:, h, :],
                                    start=(h == 0), stop=(h == G - 1))

                        def evac(yp=yp, stl=stl, ng=ng, q0=q0):
                            ys = ysp.tile([P, SC], BF16, tag="ys")
                            # vector only: a copy landing on ACT would evict
                            # the Exp table mid-attention (1.3us reload)
                            nc.vector.tensor_copy(ys, yp)
                            # store via the GpSimd SWDGE queue: keeps 128
                            # store issues off the Sync queue, and per-stl
                            # pieces land on parallel DMA engines
                            nc.gpsimd.dma_start(
                                y[q0 + stl * P:q0 + (stl + 1) * P,
                                  ng * SC:(ng + 1) * SC], ys)
                        yield evac

            def emit_attn(c, qr_t, filler):
                """Attention for chunk c; `filler` ops (outproj of c-1)
                are interleaved to keep PE busy while ACT runs exp."""
                def take(n):
                    for _ in range(n):
                        op = next(filler, None)
                        if op is None:
                            return
                        op()

                # boundary filler: outproj matmuls keep the PE fed while
                # the DVE drains the proj phase's rope-chain backlog
                take(21)
                nkt = (c + 1) * 4
                for h in range(G):
                    avp = psD.tile([P, SC], F32, tag="d", name=f"av{c}_{h}")
                    dnp = psC.tile([P, SC], F32, tag="c", name=f"dn{c}_{h}")
                    for kt in range(nkt):
                        j = kt - c * 4
                        off = P * j if j >= 0 else 0
                        ptps = psB.tile([P, SC], F32, tag="b",
                                        name=f"pt{c}_{h}_{kt}")
                        nc.tensor.matmul(
                            ptps[:, off:], KR[:, kt * P:(kt + 1) * P],
                            qr_t[:, h, off:], start=True, stop=True)
                        pt = ptp.tile([P, SC], BF16, tag="pt")
                        nc.scalar.activation(pt[:, off:], ptps[:, off:], Exp,
                                             bias=0.0, scale=1.0)
                        if j >= 0:
                            nc.vector.tensor_mul(pt[:, off:], pt[:, off:],
                                                 masks[j][:, off:])
                        nc.tensor.matmul(dnp[:, off:], ones_bf, pt[:, off:],
                                         start=(kt == 0), stop=(kt == nkt - 1))
                        nc.tensor.matmul(avp[:, off:], Vs[:, kt, :],
                                         pt[:, off:],
                                         start=(kt == 0), stop=(kt == nkt - 1))
                        take(2)
                    rcp = rcpp.tile([P, SC], F32, tag="rcp")
                    nc.vector.reciprocal_approx_fast(rcp, dnp)
                    ot = otp.tile([P, SC], BF16, tag="ot", name=f"ot{c}_{h}")
                    nc.vector.tensor_mul(ot, avp, rcp)
                    ots[(c, h)] = ot
                    take(7)

            # ---- pipelined main loop ----
            qr_next = emit_proj(0)
            empty = iter(())
            for c in range(NSC):
                qr_cur = qr_next
                filler = outproj_ops(c - 1) if c >= 1 else empty
                if c >= 1:
                    next(filler)()  # prime wog(0) DMA before proj(c+1)
                    next(filler)()  # prime wog(1)
                if c + 1 < NSC:
                    qr_next = emit_proj(c + 1)
                emit_attn(c, qr_cur, filler)
                for op in filler:  # flush leftovers
                    op()
            for op in outproj_ops(NSC - 1):
                op()

    nc.finalize()
    return nc


def shard_inputs(x, wq, wk, wv, wo, q_norm_w, k_norm_w, cos_table, sin_table,
                 positions, **_ignored):
    """Host-side sharding: returns the list of 8 per-core input maps."""
    import ml_dtypes
    bf16 = ml_dtypes.bfloat16

    x = np.asarray(x, np.float32)
    pos = np.asarray(positions).astype(np.int64)
    cos_sel = np.asarray(cos_table, np.float32)[pos]   # [S, D]
    sin_sel = np.asarray(sin_table, np.float32)[pos]
    qw = np.ascontiguousarray(
        np.asarray(q_norm_w, np.float32).reshape(D, 1))
    kw = np.ascontiguousarray(
        np.asarray(k_norm_w, np.float32).reshape(D, 1))
    # fold rotate-half's minus sign into sin rows 0..63:
    # rope(z) = z*cos + [-z2; z1]*sin = z*cos + [z2; z1]*sin_eff
    sign = np.ones((1, D), np.float32)
    sign[0, :D // 2] = -1.0
    cost = np.ascontiguousarray(cos_sel.T).astype(bf16)            # [D, S]
    sint = np.ascontiguousarray((sin_sel * sign).T).astype(bf16)
    # x as [p, ht, s]: x[s, ht*128+p]
    xT3 = np.ascontiguousarray(
        x.reshape(S, NHT, P).transpose(2, 1, 0)).astype(bf16)
    wq = np.asarray(wq, np.float32)
    wk = np.asarray(wk, np.float32)
    wv = np.asarray(wv, np.float32)
    wo = np.asarray(wo, np.float32)

    in_maps = []
    for c in range(N_CORES):
        # weight shards, [p, ht, m] with p the contraction partition
        wq_s = wq[c * G * P:(c + 1) * G * P, :].T     # [HID, 512]
        wk_s = wk[c * P:(c + 1) * P, :].T             # [HID, 128]
        wv_s = wv[c * P:(c + 1) * P, :].T
        wo_s = wo[:, c * G * P:(c + 1) * G * P].T     # [512, HID]
        m = {
            "xT": xT3,
            "wqT": np.ascontiguousarray(
                wq_s.reshape(NHT, P, G * P).transpose(1, 0, 2)).astype(bf16),
            "wkT": np.ascontiguousarray(
                wk_s.reshape(NHT, P, P).transpose(1, 0, 2)).astype(bf16),
            "wvT": np.ascontiguousarray(
                wv_s.reshape(NHT, P, P).transpose(1, 0, 2)).astype(bf16),
            "woT": np.ascontiguousarray(
                wo_s.reshape(G, P, HID).transpose(1, 0, 2)).astype(bf16),
            "cost": cost, "sint": sint, "qnw": qw, "knw": kw,
        }
        in_maps.append(m)
    return in_maps


_NC = None


def _get_nc():
    global _NC
    if _NC is None:
        _NC = build_program()
    return _NC


def run_on_device(in_maps, trace=False):
    from concourse.bass_utils import run_bass_kernel_spmd
    nc = _get_nc()
    return run_bass_kernel_spmd(nc, in_maps, list(range(N_CORES)), trace=trace)


def kernel(**inputs):
    in_maps = shard_inputs(**inputs)
    res = run_on_device(in_maps).results
    y = np.zeros((S, HID), np.float32)
    for c in range(N_CORES):
        y += np.asarray(res[c]["y"], np.float32)
    return y.reshape(1, S, HID)


# revision 34
# speedup vs baseline: 1.0308x; 1.0296x over previous
"""GQA attention prefill kernel for Trainium2 (Bass/Tile), 8-way tensor
parallel over heads.

Problem (hardcoded): B=1, S=2048, HID=4096, NH=32, KVH=8, D=128, causal
prefill with per-head RMSNorm on q/k and RoPE, positions = arange(S).

Sharding: core c owns kv-head c and q-heads 4c..4c+3. wq/wo sharded on the
head dim, wk/wv on the kv-head dim; x, rope tables replicated. Each core
computes its 4 heads' contribution through wo; the host sums the 8 partial
outputs (partials shipped as bf16, summed in fp32).

All matmul operands are bf16 (PE runs 1 cycle/row and FWL halves weight
loads; fp32r measured ~2.2 cycles/row on HW). PSUM accumulation stays fp32.
The rotate-half sign is folded into the sin table host-side; the q/k norm
weights are applied on-device as a per-partition scalar in the fused
(pre * w) * rsqrt multiply.

Weights/activations are shipped in [partition, tile, free] 3-D layouts so
every SBUF load is one strided DMA descriptor (the Sync sequencer pays
~600 ns per dma_start; v1 of this kernel lost ~250 us to descriptor issue).
Big transfers are split across a few descriptors so multiple DMA engines
run in parallel (one queue sustains only ~24 GB/s).

Structure (per chunk of 512 q positions), software-pipelined:
  proj(c+1) -> attention(c) with outproj(c-1) matmuls interleaved as PE
  filler while the scalar engine grinds exp.
Projections are head-serial (one PSUM bank at a time, x chunk resident in
SBUF) so only 2 proj banks are ever live; PSUM budget is
2 (proj+outproj) + 3 (scores) + 2 (softmax denom) + 1 (att out) = 8 banks.

RMS-norm cross-partition sums run on GpSimd (partition_all_reduce) instead
of PE ones-matmuls; reciprocals use the fast DVE approximation. The causal
diagonal k-tiles compute only the valid q columns (free-dim trim).
"""

import numpy as np

import concourse.bass as bass
import concourse.mybir as mybir
import concourse.tile as tile
from concourse import bacc
from concourse import bass_isa
from concourse.masks import make_identity

P = 128
S = 2048
HID = 4096
D = 128
G = 4            # q heads per core
NHT = HID // P   # 32 h-tiles (contraction)
SC = 512         # seq chunk
NSC = S // SC    # 4
NKT = S // P     # 16 k-tiles
EPS = 1e-6
N_CORES = 8

F32 = mybir.dt.float32
BF16 = mybir.dt.bfloat16
MULT = mybir.AluOpType.mult


def build_program():
    nc = bacc.Bacc("TRN2", target_bir_lowering=False, debug=False)

    xT = nc.dram_tensor("xT", [P, NHT, S], BF16, kind="ExternalInput").ap()
    wqT = nc.dram_tensor("wqT", [P, NHT, G * P], BF16,
                         kind="ExternalInput").ap()
    wkT = nc.dram_tensor("wkT", [P, NHT, P], BF16, kind="ExternalInput").ap()
    wvT = nc.dram_tensor("wvT", [P, NHT, P], BF16, kind="ExternalInput").ap()
    woT = nc.dram_tensor("woT", [P, G, HID], BF16, kind="ExternalInput").ap()
    cost = nc.dram_tensor("cost", [D, S], BF16, kind="ExternalInput").ap()
    sint = nc.dram_tensor("sint", [D, S], BF16, kind="ExternalInput").ap()
    qnw = nc.dram_tensor("qnw", [D, 1], F32, kind="ExternalInput").ap()
    knw = nc.dram_tensor("knw", [D, 1], F32, kind="ExternalInput").ap()
    y = nc.dram_tensor("y", [S, HID], BF16, kind="ExternalOutput").ap()

    Sqrt = mybir.ActivationFunctionType.Sqrt
    Exp = mybir.ActivationFunctionType.Exp

    with tile.TileContext(nc) as tc:
        with (
            tc.tile_pool(name="const", bufs=1) as const,
            tc.tile_pool(name="wres", bufs=1) as wres,
            tc.tile_pool(name="xp", bufs=2) as xp,
            tc.tile_pool(name="qrp", bufs=2) as qrp,
            tc.tile_pool(name="wog", bufs=3) as wogp,
            tc.tile_pool(name="scr", bufs=3) as scr,
            tc.tile_pool(name="ptp", bufs=3) as ptp,
            tc.tile_pool(name="otp", bufs=8) as otp,
            tc.tile_pool(name="rcpp", bufs=2) as rcpp,
            tc.tile_pool(name="ysp", bufs=6) as ysp,
            # PSUM: exactly 8 banks total.
            tc.tile_pool(name="psA", bufs=2, space="PSUM") as psA,  # proj+outproj
            tc.tile_pool(name="psB", bufs=3, space="PSUM") as psB,  # scores+transp
            tc.tile_pool(name="psC", bufs=2, space="PSUM") as psC,  # softmax denom
            tc.tile_pool(name="psD", bufs=1, space="PSUM") as psD,  # att out
        ):
            # ---- resident tensors (batched loads, interleaved so the
            # first projection stream unblocks early) ----
            wq_sb = wres.tile([P, NHT, G * P], BF16)
            wk_sb = wres.tile([P, NHT, P], BF16)
            wv_sb = wres.tile([P, NHT, P], BF16)
            x_tiles = [xp.tile([P, NHT, SC], BF16, tag="xall", name=f"x{c}")
                       for c in range(NSC)]
            # one DMA engine moves only ~24 GB/s, so keep pieces <=128KB and
            # let them land on parallel engines. Partition-split the very
            # first x/wv pieces so the first matmul (V stream) starts early.
            for pp in (slice(0, 64), slice(64, 128)):
                nc.sync.dma_start(x_tiles[0][pp, 0:1, :], xT[pp, 0:1, 0:SC])
                nc.sync.dma_start(wv_sb[pp, 0:2, :], wvT[pp, 0:2, :])
            nc.sync.dma_start(x_tiles[0][:, 1:2, :], xT[:, 1:2, 0:SC])
            for i in range(1, 16):
                h2 = slice(i * 2, (i + 1) * 2)
                nc.sync.dma_start(x_tiles[0][:, h2, :], xT[:, h2, 0:SC])
                if i >= 1:
                    nc.sync.dma_start(wv_sb[:, h2, :], wvT[:, h2, :])
            for i in range(8):
                h4 = slice(i * 4, (i + 1) * 4)
                nc.sync.dma_start(wk_sb[:, h4, :], wkT[:, h4, :])
                nc.sync.dma_start(wq_sb[:, h4, :], wqT[:, h4, :])
            cs = wres.tile([P, S], BF16, name="cs")
            sn = wres.tile([P, S], BF16, name="sn")
            for i in range(4):
                sl = slice(i * S // 4, (i + 1) * S // 4)
                nc.sync.dma_start(cs[:, sl], cost[:, sl])
                nc.sync.dma_start(sn[:, sl], sint[:, sl])
            qn = const.tile([P, 1], F32, name="qn")
            kn = const.tile([P, 1], F32, name="kn")
            nc.sync.dma_start(qn, qnw)
            nc.sync.dma_start(kn, knw)

            # ---- constants ----
            identity = const.tile([P, P], BF16)
            make_identity(nc, identity)
            f32tmp = const.tile([P, SC], F32)
            ones_bf = const.tile([P, P], BF16)
            nc.gpsimd.memset(f32tmp, 1.0)
            nc.vector.tensor_copy(ones_bf, f32tmp[:, 0:P])
            # causal masks for the 4 diagonal k-tiles of a q chunk:
            # keep (1.0) where q_local >= 128*j + k_local
            masks = []
            for j in range(4):
                mk = const.tile([P, SC], BF16, name=f"mask{j}")
                nc.gpsimd.memset(f32tmp, 1.0)
                nc.gpsimd.affine_select(
                    f32tmp, f32tmp, pattern=[[1, SC]],
                    compare_op=mybir.AluOpType.is_ge,
                    fill=0.0, base=-P * j, channel_multiplier=-1,
                )
                nc.vector.tensor_copy(mk, f32tmp)
                masks.append(mk)

            bias_keps = const.tile([P, 1], F32)
            nc.gpsimd.memset(bias_keps, float(P) * EPS)
            bias_qeps = const.tile([P, 1], F32)
            nc.gpsimd.memset(bias_qeps, EPS)

            KR = wres.tile([P, S], BF16, name="KR")       # roped+scaled K [d, s]
            Vs = wres.tile([P, NKT, P], BF16, name="Vs")  # V [s-in-tile, kt, d]

            # ot tiles per (chunk, head) — kept alive until outproj(c) done
            ots = {}

            def emit_proj(c):
                """Projections + rope/norm for chunk c. Head-serial: one
                PSUM bank per stream. The rope chain is pipelined one
                stream deep: stream s+1's PSUM evac is emitted before
                stream s's sqrt/rope tail, so the ACT queue never blocks
                on the GpSimd partition-reduce."""
                q0 = c * SC
                xall = x_tiles[c]
                qr_t = qrp.tile([P, G, SC], BF16, tag="qr", name=f"qr{c}")

                def evac(pj):
                    """PSUM evac + square-sum launch + rotate-half DMAs.
                    The rot DMAs ride the Activation HWDGE queue so they
                    never wait behind bulk x/w transfers on Sync's."""
                    raw = scr.tile([P, SC], BF16, tag="raw")
                    nc.scalar.copy(raw, pj)
                    rot = scr.tile([P, SC], BF16, tag="rot")
                    nc.sync.dma_start(rot[0:64], raw[64:128])
                    nc.sync.dma_start(rot[64:128], raw[0:64])
                    sqv = scr.tile([P, SC], BF16, tag="sqv")
                    nc.vector.tensor_mul(sqv, raw, raw)
                    ssq = scr.tile([P, SC], F32, tag="ssq")
                    nc.gpsimd.partition_all_reduce(
                        ssq, sqv, P, bass_isa.ReduceOp.add)
                    return raw, rot, ssq

                def finish(st, nw, bias, scale, out_ap):
                    raw, rot, ssq = st
                    nc.scalar.activation(ssq, ssq, Sqrt, bias=bias, scale=scale)
                    rkf = scr.tile([P, SC], F32, tag="rkf")
                    nc.vector.reciprocal_approx_fast(rkf, ssq)
                    t1 = scr.tile([P, SC], BF16, tag="t1")
                    nc.vector.tensor_mul(t1, rot, sn[:, q0:q0 + SC])
                    # in-place: raw = raw*cos (rot DMA read already consumed
                    # raw; WAR tracked by the framework)
                    nc.vector.tensor_mul(raw, raw, cs[:, q0:q0 + SC])
                    nc.vector.tensor_add(raw, raw, t1)
                    # out = (raw * norm_w[P,1]) * rsqrt_factor, one DVE op
                    nc.vector.scalar_tensor_tensor(
                        out_ap, raw, nw, rkf, MULT, MULT)

                # v first: its PE transposes share the psB ring with the
                # next attention phase's score tiles, so they must happen
                # early, not at the phase boundary
                pj = psA.tile([P, SC], F32, tag="a", name=f"vp{c}")
                for ht in range(NHT):
                    nc.tensor.matmul(pj, wv_sb[:, ht, :], xall[:, ht, :],
                                     start=(ht == 0), stop=(ht == NHT - 1))
                vtmp = scr.tile([P, SC], BF16, tag="vtmp")
                # ACT, not DVE: the DVE queue backs up with rope chains at
                # the end of the proj phase and would delay the transposes
                nc.scalar.copy(vtmp, pj)
                for j in range(SC // P):
                    tp = psB.tile([P, P], BF16, tag="b", name=f"tp{c}_{j}")
                    nc.tensor.transpose(tp, vtmp[:, j * P:(j + 1) * P], identity)
                    nc.vector.tensor_copy(Vs[:, c * 4 + j, :], tp)
                # k, then the 4 q heads
                pending = None
                for h in range(-1, G):
                    pj = psA.tile([P, SC], F32, tag="a", name=f"qp{c}_{h}")
                    for ht in range(NHT):
                        if h >= 0:
                            lhs = wq_sb[:, ht, h * P:(h + 1) * P]
                        else:
                            lhs = wk_sb[:, ht, :]
                        nc.tensor.matmul(pj, lhs, xall[:, ht, :],
                                         start=(ht == 0), stop=(ht == NHT - 1))
                    st = evac(pj)
                    if pending is not None:
                        pending()
                    if h >= 0:
                        pending = (lambda st=st, h=h: finish(
                            st, qn, bias_qeps, 1.0 / P, qr_t[:, h, :]))
                    else:
                        pending = (lambda st=st: finish(
                            st, kn, bias_keps, 1.0, KR[:, q0:q0 + SC]))
                    del st
                pending()
                # prefetch next chunk's x last so its Sync issues never sit
                # ahead of this chunk's latency-critical rot DMAs
                if c + 1 < NSC:
                    for i in range(16):
                        h2 = slice(i * 2, (i + 1) * 2)
                        nc.sync.dma_start(x_tiles[c + 1][:, h2, :],
                                          xT[:, h2, q0 + SC:q0 + 2 * SC])
                return qr_t

            def outproj_ops(c):
                """Generator of closures: output projection for chunk c,
                in PE-sized steps (one matmul per step). Used as filler
                between attention matmuls of chunk c+1."""
                q0 = c * SC
                wogs = {}

                def load_wog(ng):
                    wog = wogp.tile([P, G, SC], BF16, tag="wog")
                    for h in range(G):
                        nc.sync.dma_start(wog[:, h, :],
                                          woT[:, h, ng * SC:(ng + 1) * SC])
                    wogs[ng] = wog

                # first two groups' weights prefetched by the priming
                # next()-calls issued before proj(c+2) is emitted
                yield lambda: load_wog(0)
                yield lambda: load_wog(1)
                for ng in range(HID // SC):
                    wog = wogs[ng]
                    if ng + 2 < HID // SC:
                        yield lambda ng=ng: load_wog(ng + 2)
                    for stl in range(SC // P):
                        yp = psA.tile([P, SC], F32, tag="a",
                                      name=f"yp{c}_{ng}_{stl}")
                        for h in range(G):
                            yield lambda yp=yp, h=h, stl=stl, wog=wog, c=c: \
                                nc.tensor.matmul(
                                    yp, ots[(c, h)][:, stl * P:(stl + 1) * P],
                                    wog[:, h, :],
                                    start=(h == 0), stop=(h == G - 1))

                        def evac(yp=yp, stl=stl, ng=ng, q0=q0):
                            ys = ysp.tile([P, SC], BF16, tag="ys")
                            # vector only: a copy landing on ACT would evict
                            # the Exp table mid-attention (1.3us reload)
                            nc.vector.tensor_copy(ys, yp)
                            # store via the GpSimd SWDGE queue: keeps 128
                            # store issues off the Sync queue, and per-stl
                            # pieces land on parallel DMA engines
                            nc.gpsimd.dma_start(
                                y[q0 + stl * P:q0 + (stl + 1) * P,
                                  ng * SC:(ng + 1) * SC], ys)
                        yield evac

            def emit_attn(c, qr_t, filler):
                """Attention for chunk c; `filler` ops (outproj of c-1)
                are interleaved to keep PE busy while ACT runs exp."""
                def take(n):
                    for _ in range(n):
                        op = next(filler, None)
                        if op is None:
                            return
                        op()

                # boundary filler: outproj matmuls keep the PE fed while
                # the DVE drains the proj phase's rope-chain backlog
                take(21)
                nkt = (c + 1) * 4
                for h in range(G):
                    avp = psD.tile([P, SC], F32, tag="d", name=f"av{c}_{h}")
                    dnp = psC.tile([P, SC], F32, tag="c", name=f"dn{c}_{h}")
                    for kt in range(nkt):
                        j = kt - c * 4
                        off = P * j if j >= 0 else 0
                        ptps = psB.tile([P, SC], F32, tag="b",
                                        name=f"pt{c}_{h}_{kt}")
                        nc.tensor.matmul(
                            ptps[:, off:], KR[:, kt * P:(kt + 1) * P],
                            qr_t[:, h, off:], start=True, stop=True)
                        pt = ptp.tile([P, SC], BF16, tag="pt")
                        nc.scalar.activation(pt[:, off:], ptps[:, off:], Exp,
                                             bias=0.0, scale=1.0)
                        if j >= 0:
                            nc.vector.tensor_mul(pt[:, off:], pt[:, off:],
                                                 masks[j][:, off:])
                        nc.tensor.matmul(dnp[:, off:], ones_bf, pt[:, off:],
                                         start=(kt == 0), stop=(kt == nkt - 1))
                        nc.tensor.matmul(avp[:, off:], Vs[:, kt, :],
                                         pt[:, off:],
                                         start=(kt == 0), stop=(kt == nkt - 1))
                        take(2)
                    rcp = rcpp.tile([P, SC], F32, tag="rcp")
                    nc.vector.reciprocal_approx_fast(rcp, dnp)
                    ot = otp.tile([P, SC], BF16, tag="ot", name=f"ot{c}_{h}")
                    nc.vector.tensor_mul(ot, avp, rcp)
                    ots[(c, h)] = ot
                    take(7)

            # ---- pipelined main loop ----
            qr_next = emit_proj(0)
            empty = iter(())
            for c in range(NSC):
                qr_cur = qr_next
                filler = outproj_ops(c - 1) if c >= 1 else empty
                if c >= 1:
                    next(filler)()  # prime wog(0) DMA before proj(c+1)
                    next(filler)()  # prime wog(1)
                if c + 1 < NSC:
                    qr_next = emit_proj(c + 1)
                emit_attn(c, qr_cur, filler)
                for op in filler:  # flush leftovers
                    op()
            for op in outproj_ops(NSC - 1):
                op()

    nc.finalize()
    return nc


def shard_inputs(x, wq, wk, wv, wo, q_norm_w, k_norm_w, cos_table, sin_table,
                 positions, **_ignored):
    """Host-side sharding: returns the list of 8 per-core input maps."""
    import ml_dtypes
    bf16 = ml_dtypes.bfloat16

    x = np.asarray(x, np.float32)
    pos = np.asarray(positions).astype(np.int64)
    cos_sel = np.asarray(cos_table, np.float32)[pos]   # [S, D]
    sin_sel = np.asarray(sin_table, np.float32)[pos]
    qw = np.ascontiguousarray(
        np.asarray(q_norm_w, np.float32).reshape(D, 1))
    kw = np.ascontiguousarray(
        np.asarray(k_norm_w, np.float32).reshape(D, 1))
    # fold rotate-half's minus sign into sin rows 0..63:
    # rope(z) = z*cos + [-z2; z1]*sin = z*cos + [z2; z1]*sin_eff
    sign = np.ones((1, D), np.float32)
    sign[0, :D // 2] = -1.0
    cost = np.ascontiguousarray(cos_sel.T).astype(bf16)            # [D, S]
    sint = np.ascontiguousarray((sin_sel * sign).T).astype(bf16)
    # x as [p, ht, s]: x[s, ht*128+p]
    xT3 = np.ascontiguousarray(
        x.reshape(S, NHT, P).transpose(2, 1, 0)).astype(bf16)
    wq = np.asarray(wq, np.float32)
    wk = np.asarray(wk, np.float32)
    wv = np.asarray(wv, np.float32)
    wo = np.asarray(wo, np.float32)

    in_maps = []
    for c in range(N_CORES):
        # weight shards, [p, ht, m] with p the contraction partition
        wq_s = wq[c * G * P:(c + 1) * G * P, :].T     # [HID, 512]
        wk_s = wk[c * P:(c + 1) * P, :].T             # [HID, 128]
        wv_s = wv[c * P:(c + 1) * P, :].T
        wo_s = wo[:, c * G * P:(c + 1) * G * P].T     # [512, HID]
        m = {
            "xT": xT3,
            "wqT": np.ascontiguousarray(
                wq_s.reshape(NHT, P, G * P).transpose(1, 0, 2)).astype(bf16),
            "wkT": np.ascontiguousarray(
                wk_s.reshape(NHT, P, P).transpose(1, 0, 2)).astype(bf16),
            "wvT": np.ascontiguousarray(
                wv_s.reshape(NHT, P, P).transpose(1, 0, 2)).astype(bf16),
            "woT": np.ascontiguousarray(
                wo_s.reshape(G, P, HID).transpose(1, 0, 2)).astype(bf16),
            "cost": cost, "sint": sint, "qnw": qw, "knw": kw,
        }
        in_maps.append(m)
    return in_maps


_NC = None


def _get_nc():
    global _NC
    if _NC is None:
        _NC = build_program()
    return _NC


def run_on_device(in_maps, trace=False):
    from concourse.bass_utils import run_bass_kernel_spmd
    nc = _get_nc()
    return run_bass_kernel_spmd(nc, in_maps, list(range(N_CORES)), trace=trace)


def kernel(**inputs):
    in_maps = shard_inputs(**inputs)
    res = run_on_device(in_maps).results
    y = np.zeros((S, HID), np.float32)
    for c in range(N_CORES):
        y += np.asarray(res[c]["y"], np.float32)
    return y.reshape(1, S, HID)


# revision 37
# speedup vs baseline: 1.0645x; 1.0328x over previous
"""GQA attention prefill kernel for Trainium2 (Bass/Tile), 8-way tensor
parallel over heads.

Problem (hardcoded): B=1, S=2048, HID=4096, NH=32, KVH=8, D=128, causal
prefill with per-head RMSNorm on q/k and RoPE, positions = arange(S).

Sharding: core c owns kv-head c and q-heads 4c..4c+3. wq/wo sharded on the
head dim, wk/wv on the kv-head dim; x, rope tables replicated. Each core
computes its 4 heads' contribution through wo; the host sums the 8 partial
outputs (partials shipped as bf16, summed in fp32).

All matmul operands are bf16 (PE runs 1 cycle/row and FWL halves weight
loads; fp32r measured ~2.2 cycles/row on HW). PSUM accumulation stays fp32.
The rotate-half sign is folded into the sin table host-side; the q/k norm
weights are applied on-device as a per-partition scalar in the fused
(pre * w) * rsqrt multiply.

Weights/activations are shipped in [partition, tile, free] 3-D layouts so
every SBUF load is one strided DMA descriptor (the Sync sequencer pays
~600 ns per dma_start; v1 of this kernel lost ~250 us to descriptor issue).
Big transfers are split across a few descriptors so multiple DMA engines
run in parallel (one queue sustains only ~24 GB/s).

Structure (per chunk of 512 q positions), software-pipelined:
  proj(c+1) -> attention(c) with outproj(c-1) matmuls interleaved as PE
  filler while the scalar engine grinds exp.
Projections are head-serial (one PSUM bank at a time, x chunk resident in
SBUF) so only 2 proj banks are ever live; PSUM budget is
2 (proj+outproj) + 3 (scores) + 2 (softmax denom) + 1 (att out) = 8 banks.

RMS-norm cross-partition sums run on GpSimd (partition_all_reduce) instead
of PE ones-matmuls; reciprocals use the fast DVE approximation. The causal
diagonal k-tiles compute only the valid q columns (free-dim trim).
"""

import numpy as np

import concourse.bass as bass
import concourse.mybir as mybir
import concourse.tile as tile
from concourse import bacc
from concourse import bass_isa
from concourse.masks import make_identity

P = 128
S = 2048
HID = 4096
D = 128
G = 4            # q heads per core
NHT = HID // P   # 32 h-tiles (contraction)
SC = 512         # seq chunk
NSC = S // SC    # 4
NKT = S // P     # 16 k-tiles
EPS = 1e-6
N_CORES = 8

F32 = mybir.dt.float32
BF16 = mybir.dt.bfloat16
MULT = mybir.AluOpType.mult


def build_program():
    nc = bacc.Bacc("TRN2", target_bir_lowering=False, debug=False)

    xT = nc.dram_tensor("xT", [P, NHT, S], BF16, kind="ExternalInput").ap()
    wqT = nc.dram_tensor("wqT", [P, NHT, G * P], BF16,
                         kind="ExternalInput").ap()
    wkT = nc.dram_tensor("wkT", [P, NHT, P], BF16, kind="ExternalInput").ap()
    wvT = nc.dram_tensor("wvT", [P, NHT, P], BF16, kind="ExternalInput").ap()
    woT = nc.dram_tensor("woT", [P, G, HID], BF16, kind="ExternalInput").ap()
    cost = nc.dram_tensor("cost", [D, S], BF16, kind="ExternalInput").ap()
    sint = nc.dram_tensor("sint", [D, S], BF16, kind="ExternalInput").ap()
    qnw = nc.dram_tensor("qnw", [D, 1], F32, kind="ExternalInput").ap()
    knw = nc.dram_tensor("knw", [D, 1], F32, kind="ExternalInput").ap()
    y = nc.dram_tensor("y", [S, HID], BF16, kind="ExternalOutput").ap()

    Sqrt = mybir.ActivationFunctionType.Sqrt
    Exp = mybir.ActivationFunctionType.Exp

    with tile.TileContext(nc) as tc:
        with (
            tc.tile_pool(name="const", bufs=1) as const,
            tc.tile_pool(name="wres", bufs=1) as wres,
            tc.tile_pool(name="xp", bufs=2) as xp,
            tc.tile_pool(name="qrp", bufs=2) as qrp,
            tc.tile_pool(name="wog", bufs=3) as wogp,
            tc.tile_pool(name="scr", bufs=3) as scr,
            tc.tile_pool(name="ptp", bufs=3) as ptp,
            tc.tile_pool(name="otp", bufs=8) as otp,
            tc.tile_pool(name="rcpp", bufs=2) as rcpp,
            tc.tile_pool(name="ysp", bufs=6) as ysp,
            # PSUM: exactly 8 banks total.
            tc.tile_pool(name="psA", bufs=2, space="PSUM") as psA,  # proj+outproj
            tc.tile_pool(name="psB", bufs=3, space="PSUM") as psB,  # scores+transp
            tc.tile_pool(name="psC", bufs=2, space="PSUM") as psC,  # softmax denom
            tc.tile_pool(name="psD", bufs=1, space="PSUM") as psD,  # att out
        ):
            # ---- resident tensors (batched loads, interleaved so the
            # first projection stream unblocks early) ----
            wq_sb = wres.tile([P, NHT, G * P], BF16)
            wk_sb = wres.tile([P, NHT, P], BF16)
            wv_sb = wres.tile([P, NHT, P], BF16)
            x_tiles = [xp.tile([P, NHT, SC], BF16, tag="xall", name=f"x{c}")
                       for c in range(NSC)]
            # one DMA engine moves only ~24 GB/s, so keep pieces <=128KB and
            # let them land on parallel engines. Partition-split the very
            # first x/wv pieces so the first matmul (V stream) starts early.
            for pp in (slice(0, 64), slice(64, 128)):
                nc.sync.dma_start(x_tiles[0][pp, 0:1, :], xT[pp, 0:1, 0:SC])
                nc.sync.dma_start(wv_sb[pp, 0:2, :], wvT[pp, 0:2, :])
            nc.sync.dma_start(x_tiles[0][:, 1:2, :], xT[:, 1:2, 0:SC])
            for i in range(1, 16):
                h2 = slice(i * 2, (i + 1) * 2)
                nc.sync.dma_start(x_tiles[0][:, h2, :], xT[:, h2, 0:SC])
                if i >= 1:
                    nc.sync.dma_start(wv_sb[:, h2, :], wvT[:, h2, :])
            for i in range(8):
                h4 = slice(i * 4, (i + 1) * 4)
                nc.sync.dma_start(wk_sb[:, h4, :], wkT[:, h4, :])
                nc.sync.dma_start(wq_sb[:, h4, :], wqT[:, h4, :])
            cs = wres.tile([P, S], BF16, name="cs")
            sn = wres.tile([P, S], BF16, name="sn")
            for i in range(4):
                sl = slice(i * S // 4, (i + 1) * S // 4)
                nc.sync.dma_start(cs[:, sl], cost[:, sl])
                nc.sync.dma_start(sn[:, sl], sint[:, sl])
            qn = const.tile([P, 1], F32, name="qn")
            kn = const.tile([P, 1], F32, name="kn")
            nc.sync.dma_start(qn, qnw)
            nc.sync.dma_start(kn, knw)

            # ---- constants ----
            identity = const.tile([P, P], BF16)
            make_identity(nc, identity)
            f32tmp = const.tile([P, SC], F32)
            ones_bf = const.tile([P, P], BF16)
            nc.gpsimd.memset(f32tmp, 1.0)
            nc.vector.tensor_copy(ones_bf, f32tmp[:, 0:P])
            # causal masks for the 4 diagonal k-tiles of a q chunk:
            # keep (1.0) where q_local >= 128*j + k_local
            masks = []
            for j in range(4):
                mk = const.tile([P, SC], BF16, name=f"mask{j}")
                nc.gpsimd.memset(f32tmp, 1.0)
                nc.gpsimd.affine_select(
                    f32tmp, f32tmp, pattern=[[1, SC]],
                    compare_op=mybir.AluOpType.is_ge,
                    fill=0.0, base=-P * j, channel_multiplier=-1,
                )
                nc.vector.tensor_copy(mk, f32tmp)
                masks.append(mk)

            bias_keps = const.tile([P, 1], F32)
            nc.gpsimd.memset(bias_keps, float(P) * EPS)
            bias_qeps = const.tile([P, 1], F32)
            nc.gpsimd.memset(bias_qeps, EPS)

            KR = wres.tile([P, S], BF16, name="KR")       # roped+scaled K [d, s]
            Vs = wres.tile([P, NKT, P], BF16, name="Vs")  # V [s-in-tile, kt, d]

            # ot tiles per (chunk, head) — kept alive until outproj(c) done
            ots = {}

            def emit_proj(c):
                """Projections + rope/norm for chunk c. Head-serial: one
                PSUM bank per stream. The rope chain is pipelined one
                stream deep: stream s+1's PSUM evac is emitted before
                stream s's sqrt/rope tail, so the ACT queue never blocks
                on the GpSimd partition-reduce."""
                q0 = c * SC
                xall = x_tiles[c]
                qr_t = qrp.tile([P, G, SC], BF16, tag="qr", name=f"qr{c}")

                def evac(pj):
                    """PSUM evac + square-sum launch + rotate-half DMAs.
                    The rot DMAs ride the Activation HWDGE queue so they
                    never wait behind bulk x/w transfers on Sync's."""
                    raw = scr.tile([P, SC], BF16, tag="raw")
                    nc.scalar.copy(raw, pj)
                    rot = scr.tile([P, SC], BF16, tag="rot")
                    nc.sync.dma_start(rot[0:64], raw[64:128])
                    nc.sync.dma_start(rot[64:128], raw[0:64])
                    sqv = scr.tile([P, SC], BF16, tag="sqv")
                    nc.vector.tensor_mul(sqv, raw, raw)
                    ssq = scr.tile([P, SC], F32, tag="ssq")
                    nc.gpsimd.partition_all_reduce(
                        ssq, sqv, P, bass_isa.ReduceOp.add)
                    return raw, rot, ssq

                def finish(st, nw, bias, scale, out_ap):
                    raw, rot, ssq = st
                    nc.scalar.activation(ssq, ssq, Sqrt, bias=bias, scale=scale)
                    rkf = scr.tile([P, SC], F32, tag="rkf")
                    nc.vector.reciprocal_approx_fast(rkf, ssq)
                    t1 = scr.tile([P, SC], BF16, tag="t1")
                    nc.vector.tensor_mul(t1, rot, sn[:, q0:q0 + SC])
                    # in-place: raw = raw*cos (rot DMA read already consumed
                    # raw; WAR tracked by the framework)
                    nc.vector.tensor_mul(raw, raw, cs[:, q0:q0 + SC])
                    nc.vector.tensor_add(raw, raw, t1)
                    # out = (raw * norm_w[P,1]) * rsqrt_factor, one DVE op
                    nc.vector.scalar_tensor_tensor(
                        out_ap, raw, nw, rkf, MULT, MULT)

                # v first: its PE transposes share the psB ring with the
                # next attention phase's score tiles, so they must happen
                # early, not at the phase boundary
                pj = psA.tile([P, SC], F32, tag="a", name=f"vp{c}")
                for ht in range(NHT):
                    nc.tensor.matmul(pj, wv_sb[:, ht, :], xall[:, ht, :],
                                     start=(ht == 0), stop=(ht == NHT - 1))
                vtmp = scr.tile([P, SC], BF16, tag="vtmp")
                nc.vector.tensor_copy(vtmp, pj)
                for j in range(SC // P):
                    tp = psB.tile([P, P], BF16, tag="b", name=f"tp{c}_{j}")
                    nc.tensor.transpose(tp, vtmp[:, j * P:(j + 1) * P], identity)
                    nc.vector.tensor_copy(Vs[:, c * 4 + j, :], tp)
                # k, then the 4 q heads
                pending = None
                for h in range(-1, G):
                    pj = psA.tile([P, SC], F32, tag="a", name=f"qp{c}_{h}")
                    for ht in range(NHT):
                        if h >= 0:
                            lhs = wq_sb[:, ht, h * P:(h + 1) * P]
                        else:
                            lhs = wk_sb[:, ht, :]
                        nc.tensor.matmul(pj, lhs, xall[:, ht, :],
                                         start=(ht == 0), stop=(ht == NHT - 1))
                    st = evac(pj)
                    if pending is not None:
                        pending()
                    if h >= 0:
                        pending = (lambda st=st, h=h: finish(
                            st, qn, bias_qeps, 1.0 / P, qr_t[:, h, :]))
                    else:
                        pending = (lambda st=st: finish(
                            st, kn, bias_keps, 1.0, KR[:, q0:q0 + SC]))
                    del st
                pending()
                # prefetch next chunk's x last so its Sync issues never sit
                # ahead of this chunk's latency-critical rot DMAs
                if c + 1 < NSC:
                    for i in range(16):
                        h2 = slice(i * 2, (i + 1) * 2)
                        nc.sync.dma_start(x_tiles[c + 1][:, h2, :],
                                          xT[:, h2, q0 + SC:q0 + 2 * SC])
                return qr_t

            def outproj_ops(c):
                """Generator of closures: output projection for chunk c,
                in PE-sized steps (one matmul per step). Used as filler
                between attention matmuls of chunk c+1."""
                q0 = c * SC
                wogs = {}

                def load_wog(ng):
                    wog = wogp.tile([P, G, SC], BF16, tag="wog")
                    for h in range(G):
                        nc.sync.dma_start(wog[:, h, :],
                                          woT[:, h, ng * SC:(ng + 1) * SC])
                    wogs[ng] = wog

                # first two groups' weights prefetched by the priming
                # next()-calls issued before proj(c+2) is emitted
                yield lambda: load_wog(0)
                yield lambda: load_wog(1)
                for ng in range(HID // SC):
                    wog = wogs[ng]
                    if ng + 2 < HID // SC:
                        yield lambda ng=ng: load_wog(ng + 2)
                    for stl in range(SC // P):
                        yp = psA.tile([P, SC], F32, tag="a",
                                      name=f"yp{c}_{ng}_{stl}")
                        for h in range(G):
                            yield lambda yp=yp, h=h, stl=stl, wog=wog, c=c: \
                                nc.tensor.matmul(
                                    yp, ots[(c, h)][:, stl * P:(stl + 1) * P],
                                    wog[:, h, :],
                                    start=(h == 0), stop=(h == G - 1))

                        def evac(yp=yp, stl=stl, ng=ng, q0=q0):
                            ys = ysp.tile([P, SC], BF16, tag="ys")
                            # vector only: a copy landing on ACT would evict
                            # the Exp table mid-attention (1.3us reload)
                            nc.vector.tensor_copy(ys, yp)
                            # store via the GpSimd SWDGE queue: keeps 128
                            # store issues off the Sync queue, and per-stl
                            # pieces land on parallel DMA engines
                            nc.gpsimd.dma_start(
                                y[q0 + stl * P:q0 + (stl + 1) * P,
                                  ng * SC:(ng + 1) * SC], ys)
                        yield evac

            def emit_attn(c, qr_t, filler):
                """Attention for chunk c; `filler` ops (outproj of c-1)
                are interleaved to keep PE busy while ACT runs exp."""
                def take(n):
                    for _ in range(n):
                        op = next(filler, None)
                        if op is None:
                            return
                        op()

                nkt = (c + 1) * 4
                for h in range(G):
                    avp = psD.tile([P, SC], F32, tag="d", name=f"av{c}_{h}")
                    dnp = psC.tile([P, SC], F32, tag="c", name=f"dn{c}_{h}")
                    for kt in range(nkt):
                        j = kt - c * 4
                        off = P * j if j >= 0 else 0
                        ptps = psB.tile([P, SC], F32, tag="b",
                                        name=f"pt{c}_{h}_{kt}")
                        nc.tensor.matmul(
                            ptps[:, off:], KR[:, kt * P:(kt + 1) * P],
                            qr_t[:, h, off:], start=True, stop=True)
                        pt = ptp.tile([P, SC], BF16, tag="pt")
                        nc.scalar.activation(pt[:, off:], ptps[:, off:], Exp,
                                             bias=0.0, scale=1.0)
                        if j >= 0:
                            nc.vector.tensor_mul(pt[:, off:], pt[:, off:],
                                                 masks[j][:, off:])
                        nc.tensor.matmul(dnp[:, off:], ones_bf, pt[:, off:],
                                         start=(kt == 0), stop=(kt == nkt - 1))
                        nc.tensor.matmul(avp[:, off:], Vs[:, kt, :],
                                         pt[:, off:],
                                         start=(kt == 0), stop=(kt == nkt - 1))
                        take(2)
                    rcp = rcpp.tile([P, SC], F32, tag="rcp")
                    nc.vector.reciprocal_approx_fast(rcp, dnp)
                    ot = otp.tile([P, SC], BF16, tag="ot", name=f"ot{c}_{h}")
                    nc.vector.tensor_mul(ot, avp, rcp)
                    ots[(c, h)] = ot
                    take(7)

            # ---- pipelined main loop ----
            qr_next = emit_proj(0)
            empty = iter(())
            for c in range(NSC):
                qr_cur = qr_next
                filler = outproj_ops(c - 1) if c >= 1 else empty
                if c + 1 < NSC:
                    qr_next = emit_proj(c + 1)
                emit_attn(c, qr_cur, filler)
                for op in filler:  # flush leftovers
                    op()
            for op in outproj_ops(NSC - 1):
                op()

    nc.finalize()
    return nc


def shard_inputs(x, wq, wk, wv, wo, q_norm_w, k_norm_w, cos_table, sin_table,
                 positions, **_ignored):
    """Host-side sharding: returns the list of 8 per-core input maps."""
    import ml_dtypes
    bf16 = ml_dtypes.bfloat16

    x = np.asarray(x, np.float32)
    pos = np.asarray(positions).astype(np.int64)
    cos_sel = np.asarray(cos_table, np.float32)[pos]   # [S, D]
    sin_sel = np.asarray(sin_table, np.float32)[pos]
    qw = np.ascontiguousarray(
        np.asarray(q_norm_w, np.float32).reshape(D, 1))
    kw = np.ascontiguousarray(
        np.asarray(k_norm_w, np.float32).reshape(D, 1))
    # fold rotate-half's minus sign into sin rows 0..63:
    # rope(z) = z*cos + [-z2; z1]*sin = z*cos + [z2; z1]*sin_eff
    sign = np.ones((1, D), np.float32)
    sign[0, :D // 2] = -1.0
    cost = np.ascontiguousarray(cos_sel.T).astype(bf16)            # [D, S]
    sint = np.ascontiguousarray((sin_sel * sign).T).astype(bf16)
    # x as [p, ht, s]: x[s, ht*128+p]
    xT3 = np.ascontiguousarray(
        x.reshape(S, NHT, P).transpose(2, 1, 0)).astype(bf16)
    wq = np.asarray(wq, np.float32)
    wk = np.asarray(wk, np.float32)
    wv = np.asarray(wv, np.float32)
    wo = np.asarray(wo, np.float32)

    in_maps = []
    for c in range(N_CORES):
        # weight shards, [p, ht, m] with p the contraction partition
        wq_s = wq[c * G * P:(c + 1) * G * P, :].T     # [HID, 512]
        wk_s = wk[c * P:(c + 1) * P, :].T             # [HID, 128]
        wv_s = wv[c * P:(c + 1) * P, :].T
        wo_s = wo[:, c * G * P:(c + 1) * G * P].T     # [512, HID]
        m = {
            "xT": xT3,
            "wqT": np.ascontiguousarray(
                wq_s.reshape(NHT, P, G * P).transpose(1, 0, 2)).astype(bf16),
            "wkT": np.ascontiguousarray(
                wk_s.reshape(NHT, P, P).transpose(1, 0, 2)).astype(bf16),
            "wvT": np.ascontiguousarray(
                wv_s.reshape(NHT, P, P).transpose(1, 0, 2)).astype(bf16),
            "woT": np.ascontiguousarray(
                wo_s.reshape(G, P, HID).transpose(1, 0, 2)).astype(bf16),
            "cost": cost, "sint": sint, "qnw": qw, "knw": kw,
        }
        in_maps.append(m)
    return in_maps


_NC = None


def _get_nc():
    global _NC
    if _NC is None:
        _NC = build_program()
    return _NC


def run_on_device(in_maps, trace=False):
    from concourse.bass_utils import run_bass_kernel_spmd
    nc = _get_nc()
    return run_bass_kernel_spmd(nc, in_maps, list(range(N_CORES)), trace=trace)


def kernel(**inputs):
    in_maps = shard_inputs(**inputs)
    res = run_on_device(in_maps).results
    y = np.zeros((S, HID), np.float32)
    for c in range(N_CORES):
        y += np.asarray(res[c]["y"], np.float32)
    return y.reshape(1, S, HID)


# revision 39
# speedup vs baseline: 1.0700x; 1.0052x over previous
"""GQA attention prefill kernel for Trainium2 (Bass/Tile), 8-way tensor
parallel over heads.

Problem (hardcoded): B=1, S=2048, HID=4096, NH=32, KVH=8, D=128, causal
prefill with per-head RMSNorm on q/k and RoPE, positions = arange(S).

Sharding: core c owns kv-head c and q-heads 4c..4c+3. wq/wo sharded on the
head dim, wk/wv on the kv-head dim; x, rope tables replicated. Each core
computes its 4 heads' contribution through wo; the host sums the 8 partial
outputs (partials shipped as bf16, summed in fp32).

All matmul operands are bf16 (PE runs 1 cycle/row and FWL halves weight
loads; fp32r measured ~2.2 cycles/row on HW). PSUM accumulation stays fp32.
The rotate-half sign is folded into the sin table host-side; the q/k norm
weights are applied on-device as a per-partition scalar in the fused
(pre * w) * rsqrt multiply.

Weights/activations are shipped in [partition, tile, free] 3-D layouts so
every SBUF load is one strided DMA descriptor (the Sync sequencer pays
~600 ns per dma_start; v1 of this kernel lost ~250 us to descriptor issue).
Big transfers are split across a few descriptors so multiple DMA engines
run in parallel (one queue sustains only ~24 GB/s).

Structure (per chunk of 512 q positions), software-pipelined:
  proj(c+1) -> attention(c) with outproj(c-1) matmuls interleaved as PE
  filler while the scalar engine grinds exp.
Projections are head-serial (one PSUM bank at a time, x chunk resident in
SBUF) so only 2 proj banks are ever live; PSUM budget is
2 (proj+outproj) + 3 (scores) + 2 (softmax denom) + 1 (att out) = 8 banks.

RMS-norm cross-partition sums run on GpSimd (partition_all_reduce) instead
of PE ones-matmuls; reciprocals use the fast DVE approximation. The causal
diagonal k-tiles compute only the valid q columns (free-dim trim).
"""

import numpy as np

import concourse.bass as bass
import concourse.mybir as mybir
import concourse.tile as tile
from concourse import bacc
from concourse import bass_isa
from concourse.masks import make_identity

P = 128
S = 2048
HID = 4096
D = 128
G = 4            # q heads per core
NHT = HID // P   # 32 h-tiles (contraction)
SC = 512         # seq chunk
NSC = S // SC    # 4
NKT = S // P     # 16 k-tiles
EPS = 1e-6
N_CORES = 8

F32 = mybir.dt.float32
BF16 = mybir.dt.bfloat16
MULT = mybir.AluOpType.mult


def build_program():
    nc = bacc.Bacc("TRN2", target_bir_lowering=False, debug=False)

    xT = nc.dram_tensor("xT", [P, NHT, S], BF16, kind="ExternalInput").ap()
    wqT = nc.dram_tensor("wqT", [P, NHT, G * P], BF16,
                         kind="ExternalInput").ap()
    wkT = nc.dram_tensor("wkT", [P, NHT, P], BF16, kind="ExternalInput").ap()
    wvT = nc.dram_tensor("wvT", [P, NHT, P], BF16, kind="ExternalInput").ap()
    woT = nc.dram_tensor("woT", [P, G, HID], BF16, kind="ExternalInput").ap()
    cost = nc.dram_tensor("cost", [D, S], BF16, kind="ExternalInput").ap()
    sint = nc.dram_tensor("sint", [D, S], BF16, kind="ExternalInput").ap()
    qnw = nc.dram_tensor("qnw", [D, 1], F32, kind="ExternalInput").ap()
    knw = nc.dram_tensor("knw", [D, 1], F32, kind="ExternalInput").ap()
    y = nc.dram_tensor("y", [S, HID], BF16, kind="ExternalOutput").ap()

    Sqrt = mybir.ActivationFunctionType.Sqrt
    Exp = mybir.ActivationFunctionType.Exp

    with tile.TileContext(nc) as tc:
        with (
            tc.tile_pool(name="const", bufs=1) as const,
            tc.tile_pool(name="wres", bufs=1) as wres,
            tc.tile_pool(name="xp", bufs=2) as xp,
            tc.tile_pool(name="qrp", bufs=2) as qrp,
            tc.tile_pool(name="wog", bufs=3) as wogp,
            tc.tile_pool(name="scr", bufs=3) as scr,
            tc.tile_pool(name="ptp", bufs=4) as ptp,
            tc.tile_pool(name="otp", bufs=8) as otp,
            tc.tile_pool(name="rcpp", bufs=2) as rcpp,
            tc.tile_pool(name="ysp", bufs=6) as ysp,
            # PSUM: exactly 8 banks total.
            tc.tile_pool(name="psA", bufs=2, space="PSUM") as psA,  # proj+outproj
            tc.tile_pool(name="psB", bufs=3, space="PSUM") as psB,  # scores+transp
            tc.tile_pool(name="psC", bufs=2, space="PSUM") as psC,  # softmax denom
            tc.tile_pool(name="psD", bufs=1, space="PSUM") as psD,  # att out
        ):
            # ---- resident tensors (batched loads, interleaved so the
            # first projection stream unblocks early) ----
            wq_sb = wres.tile([P, NHT, G * P], BF16)
            wk_sb = wres.tile([P, NHT, P], BF16)
            wv_sb = wres.tile([P, NHT, P], BF16)
            x_tiles = [xp.tile([P, NHT, SC], BF16, tag="xall", name=f"x{c}")
                       for c in range(NSC)]
            # one DMA engine moves only ~24 GB/s, so keep pieces <=128KB and
            # let them land on parallel engines. Partition-split the very
            # first x/wv pieces so the first matmul (V stream) starts early.
            for pp in (slice(0, 64), slice(64, 128)):
                nc.sync.dma_start(x_tiles[0][pp, 0:1, :], xT[pp, 0:1, 0:SC])
                nc.sync.dma_start(wv_sb[pp, 0:2, :], wvT[pp, 0:2, :])
            nc.sync.dma_start(x_tiles[0][:, 1:2, :], xT[:, 1:2, 0:SC])
            for i in range(1, 16):
                h2 = slice(i * 2, (i + 1) * 2)
                nc.sync.dma_start(x_tiles[0][:, h2, :], xT[:, h2, 0:SC])
                if i >= 1:
                    nc.sync.dma_start(wv_sb[:, h2, :], wvT[:, h2, :])
            for i in range(8):
                h4 = slice(i * 4, (i + 1) * 4)
                nc.sync.dma_start(wk_sb[:, h4, :], wkT[:, h4, :])
                nc.sync.dma_start(wq_sb[:, h4, :], wqT[:, h4, :])
            cs = wres.tile([P, S], BF16, name="cs")
            sn = wres.tile([P, S], BF16, name="sn")
            for i in range(4):
                sl = slice(i * S // 4, (i + 1) * S // 4)
                nc.sync.dma_start(cs[:, sl], cost[:, sl])
                nc.sync.dma_start(sn[:, sl], sint[:, sl])
            qn = const.tile([P, 1], F32, name="qn")
            kn = const.tile([P, 1], F32, name="kn")
            nc.sync.dma_start(qn, qnw)
            nc.sync.dma_start(kn, knw)

            # ---- constants ----
            identity = const.tile([P, P], BF16)
            make_identity(nc, identity)
            f32tmp = const.tile([P, SC], F32)
            ones_bf = const.tile([P, P], BF16)
            nc.gpsimd.memset(f32tmp, 1.0)
            nc.vector.tensor_copy(ones_bf, f32tmp[:, 0:P])
            # causal masks for the 4 diagonal k-tiles of a q chunk:
            # keep (1.0) where q_local >= 128*j + k_local
            masks = []
            for j in range(4):
                mk = const.tile([P, SC], BF16, name=f"mask{j}")
                nc.gpsimd.memset(f32tmp, 1.0)
                nc.gpsimd.affine_select(
                    f32tmp, f32tmp, pattern=[[1, SC]],
                    compare_op=mybir.AluOpType.is_ge,
                    fill=0.0, base=-P * j, channel_multiplier=-1,
                )
                nc.vector.tensor_copy(mk, f32tmp)
                masks.append(mk)

            bias_keps = const.tile([P, 1], F32)
            nc.gpsimd.memset(bias_keps, float(P) * EPS)
            bias_qeps = const.tile([P, 1], F32)
            nc.gpsimd.memset(bias_qeps, EPS)

            KR = wres.tile([P, S], BF16, name="KR")       # roped+scaled K [d, s]
            Vs = wres.tile([P, NKT, P], BF16, name="Vs")  # V [s-in-tile, kt, d]

            # ot tiles per (chunk, head) — kept alive until outproj(c) done
            ots = {}

            def emit_proj(c):
                """Projections + rope/norm for chunk c. Head-serial: one
                PSUM bank per stream. The rope chain is pipelined one
                stream deep: stream s+1's PSUM evac is emitted before
                stream s's sqrt/rope tail, so the ACT queue never blocks
                on the GpSimd partition-reduce."""
                q0 = c * SC
                xall = x_tiles[c]
                qr_t = qrp.tile([P, G, SC], BF16, tag="qr", name=f"qr{c}")

                def evac(pj):
                    """PSUM evac + square-sum launch + rotate-half DMAs.
                    The rot DMAs ride the Activation HWDGE queue so they
                    never wait behind bulk x/w transfers on Sync's."""
                    raw = scr.tile([P, SC], BF16, tag="raw")
                    nc.scalar.copy(raw, pj)
                    rot = scr.tile([P, SC], BF16, tag="rot")
                    nc.sync.dma_start(rot[0:64], raw[64:128])
                    nc.sync.dma_start(rot[64:128], raw[0:64])
                    sqv = scr.tile([P, SC], BF16, tag="sqv")
                    nc.vector.tensor_mul(sqv, raw, raw)
                    ssq = scr.tile([P, SC], F32, tag="ssq")
                    nc.gpsimd.partition_all_reduce(
                        ssq, sqv, P, bass_isa.ReduceOp.add)
                    return raw, rot, ssq

                def finish(st, nw, bias, scale, out_ap):
                    raw, rot, ssq = st
                    nc.scalar.activation(ssq, ssq, Sqrt, bias=bias, scale=scale)
                    rkf = scr.tile([P, SC], F32, tag="rkf")
                    nc.vector.reciprocal_approx_fast(rkf, ssq)
                    t1 = scr.tile([P, SC], BF16, tag="t1")
                    nc.vector.tensor_mul(t1, rot, sn[:, q0:q0 + SC])
                    # in-place: raw = raw*cos (rot DMA read already consumed
                    # raw; WAR tracked by the framework)
                    nc.vector.tensor_mul(raw, raw, cs[:, q0:q0 + SC])
                    nc.vector.tensor_add(raw, raw, t1)
                    # out = (raw * norm_w[P,1]) * rsqrt_factor, one DVE op
                    nc.vector.scalar_tensor_tensor(
                        out_ap, raw, nw, rkf, MULT, MULT)

                # v first: its PE transposes share the psB ring with the
                # next attention phase's score tiles, so they must happen
                # early, not at the phase boundary
                pj = psA.tile([P, SC], F32, tag="a", name=f"vp{c}")
                for ht in range(NHT):
                    nc.tensor.matmul(pj, wv_sb[:, ht, :], xall[:, ht, :],
                                     start=(ht == 0), stop=(ht == NHT - 1))
                vtmp = scr.tile([P, SC], BF16, tag="vtmp")
                nc.vector.tensor_copy(vtmp, pj)
                for j in range(SC // P):
                    tp = psB.tile([P, P], BF16, tag="b", name=f"tp{c}_{j}")
                    nc.tensor.transpose(tp, vtmp[:, j * P:(j + 1) * P], identity)
                    nc.vector.tensor_copy(Vs[:, c * 4 + j, :], tp)
                # k, then the 4 q heads
                pending = None
                for h in range(-1, G):
                    pj = psA.tile([P, SC], F32, tag="a", name=f"qp{c}_{h}")
                    for ht in range(NHT):
                        if h >= 0:
                            lhs = wq_sb[:, ht, h * P:(h + 1) * P]
                        else:
                            lhs = wk_sb[:, ht, :]
                        nc.tensor.matmul(pj, lhs, xall[:, ht, :],
                                         start=(ht == 0), stop=(ht == NHT - 1))
                    st = evac(pj)
                    if pending is not None:
                        pending()
                    if h >= 0:
                        pending = (lambda st=st, h=h: finish(
                            st, qn, bias_qeps, 1.0 / P, qr_t[:, h, :]))
                    else:
                        pending = (lambda st=st: finish(
                            st, kn, bias_keps, 1.0, KR[:, q0:q0 + SC]))
                    del st
                pending()
                # prefetch next chunk's x last so its Sync issues never sit
                # ahead of this chunk's latency-critical rot DMAs
                if c + 1 < NSC:
                    for i in range(16):
                        h2 = slice(i * 2, (i + 1) * 2)
                        nc.sync.dma_start(x_tiles[c + 1][:, h2, :],
                                          xT[:, h2, q0 + SC:q0 + 2 * SC])
                return qr_t

            def outproj_ops(c):
                """Generator of closures: output projection for chunk c,
                in PE-sized steps (one matmul per step). Used as filler
                between attention matmuls of chunk c+1."""
                q0 = c * SC
                wogs = {}

                def load_wog(ng):
                    wog = wogp.tile([P, G, SC], BF16, tag="wog")
                    for h in range(G):
                        nc.sync.dma_start(wog[:, h, :],
                                          woT[:, h, ng * SC:(ng + 1) * SC])
                    wogs[ng] = wog

                # first two groups' weights prefetched by the priming
                # next()-calls issued before proj(c+2) is emitted
                yield lambda: load_wog(0)
                yield lambda: load_wog(1)
                for ng in range(HID // SC):
                    wog = wogs[ng]
                    if ng + 2 < HID // SC:
                        yield lambda ng=ng: load_wog(ng + 2)
                    for stl in range(SC // P):
                        yp = psA.tile([P, SC], F32, tag="a",
                                      name=f"yp{c}_{ng}_{stl}")
                        for h in range(G):
                            yield lambda yp=yp, h=h, stl=stl, wog=wog, c=c: \
                                nc.tensor.matmul(
                                    yp, ots[(c, h)][:, stl * P:(stl + 1) * P],
                                    wog[:, h, :],
                                    start=(h == 0), stop=(h == G - 1))

                        def evac(yp=yp, stl=stl, ng=ng, q0=q0):
                            ys = ysp.tile([P, SC], BF16, tag="ys")
                            # vector only: a copy landing on ACT would evict
                            # the Exp table mid-attention (1.3us reload)
                            nc.vector.tensor_copy(ys, yp)
                            # store via the GpSimd SWDGE queue: keeps 128
                            # store issues off the Sync queue, and per-stl
                            # pieces land on parallel DMA engines
                            nc.gpsimd.dma_start(
                                y[q0 + stl * P:q0 + (stl + 1) * P,
                                  ng * SC:(ng + 1) * SC], ys)
                        yield evac

            def emit_attn(c, qr_t, filler):
                """Attention for chunk c; `filler` ops (outproj of c-1)
                are interleaved to keep PE busy while ACT runs exp."""
                def take(n):
                    for _ in range(n):
                        op = next(filler, None)
                        if op is None:
                            return
                        op()

                nkt = (c + 1) * 4
                for h in range(G):
                    avp = psD.tile([P, SC], F32, tag="d", name=f"av{c}_{h}")
                    dnp = psC.tile([P, SC], F32, tag="c", name=f"dn{c}_{h}")

                    def score_tile(kt, off):
                        ptps = psB.tile([P, SC], F32, tag="b",
                                        name=f"pt{c}_{h}_{kt}")
                        nc.tensor.matmul(
                            ptps[:, off:], KR[:, kt * P:(kt + 1) * P],
                            qr_t[:, h, off:], start=True, stop=True)
                        pt = ptp.tile([P, SC], BF16, tag="pt")
                        nc.scalar.activation(pt[:, off:], ptps[:, off:], Exp,
                                             bias=0.0, scale=1.0)
                        return pt

                    # full k-tiles, in pairs: one denominator matmul per
                    # pair (the pair-sum is a cheap bf16 DVE add)
                    for base in range(0, c * 4, 2):
                        pts = []
                        for kk in range(2):
                            kt = base + kk
                            pt = score_tile(kt, 0)
                            nc.tensor.matmul(avp, Vs[:, kt, :], pt,
                                             start=(kt == 0), stop=False)
                            pts.append(pt)
                            take(2)
                        psum2 = ptp.tile([P, SC], BF16, tag="pt")
                        nc.vector.tensor_add(psum2, pts[0], pts[1])
                        nc.tensor.matmul(dnp, ones_bf, psum2,
                                         start=(base == 0), stop=False)
                        take(1)
                    # diagonal k-tiles: causal-trimmed to valid q columns
                    for j in range(4):
                        kt = c * 4 + j
                        off = P * j
                        pt = score_tile(kt, off)
                        nc.vector.tensor_mul(pt[:, off:], pt[:, off:],
                                             masks[j][:, off:])
                        nc.tensor.matmul(dnp[:, off:], ones_bf, pt[:, off:],
                                         start=(kt == 0), stop=(j == 3))
                        nc.tensor.matmul(avp[:, off:], Vs[:, kt, :],
                                         pt[:, off:],
                                         start=(kt == 0), stop=(j == 3))
                        take(2)
                    rcp = rcpp.tile([P, SC], F32, tag="rcp")
                    nc.vector.reciprocal_approx_fast(rcp, dnp)
                    ot = otp.tile([P, SC], BF16, tag="ot", name=f"ot{c}_{h}")
                    nc.vector.tensor_mul(ot, avp, rcp)
                    ots[(c, h)] = ot
                    take(7)

            # ---- pipelined main loop ----
            qr_next = emit_proj(0)
            empty = iter(())
            for c in range(NSC):
                qr_cur = qr_next
                filler = outproj_ops(c - 1) if c >= 1 else empty
                if c + 1 < NSC:
                    qr_next = emit_proj(c + 1)
                emit_attn(c, qr_cur, filler)
                for op in filler:  # flush leftovers
                    op()
            for op in outproj_ops(NSC - 1):
                op()

    nc.finalize()
    return nc


def shard_inputs(x, wq, wk, wv, wo, q_norm_w, k_norm_w, cos_table, sin_table,
                 positions, **_ignored):
    """Host-side sharding: returns the list of 8 per-core input maps."""
    import ml_dtypes
    bf16 = ml_dtypes.bfloat16

    x = np.asarray(x, np.float32)
    pos = np.asarray(positions).astype(np.int64)
    cos_sel = np.asarray(cos_table, np.float32)[pos]   # [S, D]
    sin_sel = np.asarray(sin_table, np.float32)[pos]
    qw = np.ascontiguousarray(
        np.asarray(q_norm_w, np.float32).reshape(D, 1))
    kw = np.ascontiguousarray(
        np.asarray(k_norm_w, np.float32).reshape(D, 1))
    # fold rotate-half's minus sign into sin rows 0..63:
    # rope(z) = z*cos + [-z2; z1]*sin = z*cos + [z2; z1]*sin_eff
    sign = np.ones((1, D), np.float32)
    sign[0, :D // 2] = -1.0
    cost = np.ascontiguousarray(cos_sel.T).astype(bf16)            # [D, S]
    sint = np.ascontiguousarray((sin_sel * sign).T).astype(bf16)
    # x as [p, ht, s]: x[s, ht*128+p]
    xT3 = np.ascontiguousarray(
        x.reshape(S, NHT, P).transpose(2, 1, 0)).astype(bf16)
    wq = np.asarray(wq, np.float32)
    wk = np.asarray(wk, np.float32)
    wv = np.asarray(wv, np.float32)
    wo = np.asarray(wo, np.float32)

    in_maps = []
    for c in range(N_CORES):
        # weight shards, [p, ht, m] with p the contraction partition
        wq_s = wq[c * G * P:(c + 1) * G * P, :].T     # [HID, 512]
        wk_s = wk[c * P:(c + 1) * P, :].T             # [HID, 128]
        wv_s = wv[c * P:(c + 1) * P, :].T
        wo_s = wo[:, c * G * P:(c + 1) * G * P].T     # [512, HID]
        m = {
            "xT": xT3,
            "wqT": np.ascontiguousarray(
                wq_s.reshape(NHT, P, G * P).transpose(1, 0, 2)).astype(bf16),
            "wkT": np.ascontiguousarray(
                wk_s.reshape(NHT, P, P).transpose(1, 0, 2)).astype(bf16),
            "wvT": np.ascontiguousarray(
                wv_s.reshape(NHT, P, P).transpose(1, 0, 2)).astype(bf16),
            "woT": np.ascontiguousarray(
                wo_s.reshape(G, P, HID).transpose(1, 0, 2)).astype(bf16),
            "cost": cost, "sint": sint, "qnw": qw, "knw": kw,
        }
        in_maps.append(m)
    return in_maps


_NC = None


def _get_nc():
    global _NC
    if _NC is None:
        _NC = build_program()
    return _NC


def run_on_device(in_maps, trace=False):
    from concourse.bass_utils import run_bass_kernel_spmd
    nc = _get_nc()
    return run_bass_kernel_spmd(nc, in_maps, list(range(N_CORES)), trace=trace)


def kernel(**inputs):
    in_maps = shard_inputs(**inputs)
    res = run_on_device(in_maps).results
    y = np.zeros((S, HID), np.float32)
    for c in range(N_CORES):
        y += np.asarray(res[c]["y"], np.float32)
    return y.reshape(1, S, HID)


# revision 41
# speedup vs baseline: 1.0858x; 1.0147x over previous
"""GQA attention prefill kernel for Trainium2 (Bass/Tile), 8-way tensor
parallel over heads.

Problem (hardcoded): B=1, S=2048, HID=4096, NH=32, KVH=8, D=128, causal
prefill with per-head RMSNorm on q/k and RoPE, positions = arange(S).

Sharding: core c owns kv-head c and q-heads 4c..4c+3. wq/wo sharded on the
head dim, wk/wv on the kv-head dim; x, rope tables replicated. Each core
computes its 4 heads' contribution through wo; the host sums the 8 partial
outputs (partials shipped as bf16, summed in fp32).

All matmul operands are bf16 (PE runs 1 cycle/row and FWL halves weight
loads; fp32r measured ~2.2 cycles/row on HW). PSUM accumulation stays fp32.
The rotate-half sign is folded into the sin table host-side; the q/k norm
weights are applied on-device as a per-partition scalar in the fused
(pre * w) * rsqrt multiply.

Weights/activations are shipped in [partition, tile, free] 3-D layouts so
every SBUF load is one strided DMA descriptor (the Sync sequencer pays
~600 ns per dma_start; v1 of this kernel lost ~250 us to descriptor issue).
Big transfers are split across a few descriptors so multiple DMA engines
run in parallel (one queue sustains only ~24 GB/s).

Structure (per chunk of 512 q positions), software-pipelined:
  proj(c+1) -> attention(c) with outproj(c-1) matmuls interleaved as PE
  filler while the scalar engine grinds exp.
Projections are head-serial (one PSUM bank at a time, x chunk resident in
SBUF) so only 2 proj banks are ever live; PSUM budget is
2 (proj+outproj) + 3 (scores) + 2 (softmax denom) + 1 (att out) = 8 banks.

RMS-norm cross-partition sums run on GpSimd (partition_all_reduce) instead
of PE ones-matmuls; reciprocals use the fast DVE approximation. The causal
diagonal k-tiles compute only the valid q columns (free-dim trim).
"""

import numpy as np

import concourse.bass as bass
import concourse.mybir as mybir
import concourse.tile as tile
from concourse import bacc
from concourse import bass_isa
from concourse.masks import make_identity

P = 128
S = 2048
HID = 4096
D = 128
G = 4            # q heads per core
NHT = HID // P   # 32 h-tiles (contraction)
SC = 512         # seq chunk
NSC = S // SC    # 4
NKT = S // P     # 16 k-tiles
EPS = 1e-6
N_CORES = 8

F32 = mybir.dt.float32
BF16 = mybir.dt.bfloat16
MULT = mybir.AluOpType.mult


def build_program():
    nc = bacc.Bacc("TRN2", target_bir_lowering=False, debug=False)

    xT = nc.dram_tensor("xT", [P, NHT, S], BF16, kind="ExternalInput").ap()
    wqT = nc.dram_tensor("wqT", [P, NHT, G * P], BF16,
                         kind="ExternalInput").ap()
    wkT = nc.dram_tensor("wkT", [P, NHT, P], BF16, kind="ExternalInput").ap()
    wvT = nc.dram_tensor("wvT", [P, NHT, P], BF16, kind="ExternalInput").ap()
    woT = nc.dram_tensor("woT", [P, G, HID], BF16, kind="ExternalInput").ap()
    cost = nc.dram_tensor("cost", [D, S], BF16, kind="ExternalInput").ap()
    sint = nc.dram_tensor("sint", [D, S], BF16, kind="ExternalInput").ap()
    qnw = nc.dram_tensor("qnw", [D, 1], F32, kind="ExternalInput").ap()
    knw = nc.dram_tensor("knw", [D, 1], F32, kind="ExternalInput").ap()
    y = nc.dram_tensor("y", [S, HID], BF16, kind="ExternalOutput").ap()

    Sqrt = mybir.ActivationFunctionType.Sqrt
    Exp = mybir.ActivationFunctionType.Exp

    with tile.TileContext(nc) as tc:
        with (
            tc.tile_pool(name="const", bufs=1) as const,
            tc.tile_pool(name="wres", bufs=1) as wres,
            tc.tile_pool(name="xp", bufs=2) as xp,
            tc.tile_pool(name="qrp", bufs=2) as qrp,
            tc.tile_pool(name="wog", bufs=3) as wogp,
            tc.tile_pool(name="scr", bufs=3) as scr,
            tc.tile_pool(name="ptp", bufs=4) as ptp,
            tc.tile_pool(name="otp", bufs=8) as otp,
            tc.tile_pool(name="rcpp", bufs=2) as rcpp,
            tc.tile_pool(name="ysp", bufs=6) as ysp,
            # PSUM: exactly 8 banks total.
            tc.tile_pool(name="psA", bufs=2, space="PSUM") as psA,  # proj+outproj
            tc.tile_pool(name="psB", bufs=3, space="PSUM") as psB,  # scores
            tc.tile_pool(name="psC", bufs=1, space="PSUM") as psC,  # softmax denom
            tc.tile_pool(name="psD", bufs=1, space="PSUM") as psD,  # att out
            # transposes get their own bank: sharing a ring with the score
            # tiles made each attention phase wait for the NEXT chunk's
            # V-transpose chain (~7us stall per chunk boundary)
            tc.tile_pool(name="psT", bufs=1, space="PSUM") as psT,
        ):
            # ---- resident tensors (batched loads, interleaved so the
            # first projection stream unblocks early) ----
            wq_sb = wres.tile([P, NHT, G * P], BF16)
            wk_sb = wres.tile([P, NHT, P], BF16)
            wv_sb = wres.tile([P, NHT, P], BF16)
            x_tiles = [xp.tile([P, NHT, SC], BF16, tag="xall", name=f"x{c}")
                       for c in range(NSC)]
            # one DMA engine moves only ~24 GB/s, so keep pieces <=128KB and
            # let them land on parallel engines. Partition-split the very
            # first x/wv pieces so the first matmul (V stream) starts early.
            for pp in (slice(0, 64), slice(64, 128)):
                nc.sync.dma_start(x_tiles[0][pp, 0:1, :], xT[pp, 0:1, 0:SC])
                nc.sync.dma_start(wv_sb[pp, 0:2, :], wvT[pp, 0:2, :])
            nc.sync.dma_start(x_tiles[0][:, 1:2, :], xT[:, 1:2, 0:SC])
            for i in range(1, 16):
                h2 = slice(i * 2, (i + 1) * 2)
                nc.sync.dma_start(x_tiles[0][:, h2, :], xT[:, h2, 0:SC])
                if i >= 1:
                    nc.sync.dma_start(wv_sb[:, h2, :], wvT[:, h2, :])
            for i in range(8):
                h4 = slice(i * 4, (i + 1) * 4)
                nc.sync.dma_start(wk_sb[:, h4, :], wkT[:, h4, :])
                nc.sync.dma_start(wq_sb[:, h4, :], wqT[:, h4, :])
            cs = wres.tile([P, S], BF16, name="cs")
            sn = wres.tile([P, S], BF16, name="sn")
            for i in range(4):
                sl = slice(i * S // 4, (i + 1) * S // 4)
                nc.sync.dma_start(cs[:, sl], cost[:, sl])
                nc.sync.dma_start(sn[:, sl], sint[:, sl])
            qn = const.tile([P, 1], F32, name="qn")
            kn = const.tile([P, 1], F32, name="kn")
            nc.sync.dma_start(qn, qnw)
            nc.sync.dma_start(kn, knw)

            # ---- constants ----
            identity = const.tile([P, P], BF16)
            make_identity(nc, identity)
            f32tmp = const.tile([P, SC], F32)
            ones_bf = const.tile([P, P], BF16)
            nc.gpsimd.memset(f32tmp, 1.0)
            nc.vector.tensor_copy(ones_bf, f32tmp[:, 0:P])
            # causal masks for the 4 diagonal k-tiles of a q chunk:
            # keep (1.0) where q_local >= 128*j + k_local
            masks = []
            for j in range(4):
                mk = const.tile([P, SC], BF16, name=f"mask{j}")
                nc.gpsimd.memset(f32tmp, 1.0)
                nc.gpsimd.affine_select(
                    f32tmp, f32tmp, pattern=[[1, SC]],
                    compare_op=mybir.AluOpType.is_ge,
                    fill=0.0, base=-P * j, channel_multiplier=-1,
                )
                nc.vector.tensor_copy(mk, f32tmp)
                masks.append(mk)

            bias_keps = const.tile([P, 1], F32)
            nc.gpsimd.memset(bias_keps, float(P) * EPS)
            bias_qeps = const.tile([P, 1], F32)
            nc.gpsimd.memset(bias_qeps, EPS)

            KR = wres.tile([P, S], BF16, name="KR")       # roped+scaled K [d, s]
            Vs = wres.tile([P, NKT, P], BF16, name="Vs")  # V [s-in-tile, kt, d]

            # ot tiles per (chunk, head) — kept alive until outproj(c) done
            ots = {}

            def emit_proj(c):
                """Projections + rope/norm for chunk c. Head-serial: one
                PSUM bank per stream. The rope chain is pipelined one
                stream deep: stream s+1's PSUM evac is emitted before
                stream s's sqrt/rope tail, so the ACT queue never blocks
                on the GpSimd partition-reduce."""
                q0 = c * SC
                xall = x_tiles[c]
                qr_t = qrp.tile([P, G, SC], BF16, tag="qr", name=f"qr{c}")

                def evac(pj):
                    """PSUM evac + square-sum launch + rotate-half DMAs.
                    The rot DMAs ride the Activation HWDGE queue so they
                    never wait behind bulk x/w transfers on Sync's."""
                    raw = scr.tile([P, SC], BF16, tag="raw")
                    nc.scalar.copy(raw, pj)
                    rot = scr.tile([P, SC], BF16, tag="rot")
                    nc.sync.dma_start(rot[0:64], raw[64:128])
                    nc.sync.dma_start(rot[64:128], raw[0:64])
                    sqv = scr.tile([P, SC], BF16, tag="sqv")
                    nc.vector.tensor_mul(sqv, raw, raw)
                    ssq = scr.tile([P, SC], F32, tag="ssq")
                    nc.gpsimd.partition_all_reduce(
                        ssq, sqv, P, bass_isa.ReduceOp.add)
                    return raw, rot, ssq

                def finish(st, nw, bias, scale, out_ap):
                    raw, rot, ssq = st
                    nc.scalar.activation(ssq, ssq, Sqrt, bias=bias, scale=scale)
                    rkf = scr.tile([P, SC], F32, tag="rkf")
                    nc.vector.reciprocal_approx_fast(rkf, ssq)
                    t1 = scr.tile([P, SC], BF16, tag="t1")
                    nc.vector.tensor_mul(t1, rot, sn[:, q0:q0 + SC])
                    # in-place: raw = raw*cos (rot DMA read already consumed
                    # raw; WAR tracked by the framework)
                    nc.vector.tensor_mul(raw, raw, cs[:, q0:q0 + SC])
                    nc.vector.tensor_add(raw, raw, t1)
                    # out = (raw * norm_w[P,1]) * rsqrt_factor, one DVE op
                    nc.vector.scalar_tensor_tensor(
                        out_ap, raw, nw, rkf, MULT, MULT)

                # v first: its PE transposes share the psB ring with the
                # next attention phase's score tiles, so they must happen
                # early, not at the phase boundary
                pj = psA.tile([P, SC], F32, tag="a", name=f"vp{c}")
                for ht in range(NHT):
                    nc.tensor.matmul(pj, wv_sb[:, ht, :], xall[:, ht, :],
                                     start=(ht == 0), stop=(ht == NHT - 1))
                vtmp = scr.tile([P, SC], BF16, tag="vtmp")
                nc.vector.tensor_copy(vtmp, pj)
                for j in range(SC // P):
                    tp = psT.tile([P, P], BF16, tag="t", name=f"tp{c}_{j}")
                    nc.tensor.transpose(tp, vtmp[:, j * P:(j + 1) * P], identity)
                    nc.vector.tensor_copy(Vs[:, c * 4 + j, :], tp)
                # k, then the 4 q heads
                pending = None
                for h in range(-1, G):
                    pj = psA.tile([P, SC], F32, tag="a", name=f"qp{c}_{h}")
                    for ht in range(NHT):
                        if h >= 0:
                            lhs = wq_sb[:, ht, h * P:(h + 1) * P]
                        else:
                            lhs = wk_sb[:, ht, :]
                        nc.tensor.matmul(pj, lhs, xall[:, ht, :],
                                         start=(ht == 0), stop=(ht == NHT - 1))
                    st = evac(pj)
                    if pending is not None:
                        pending()
                    if h >= 0:
                        pending = (lambda st=st, h=h: finish(
                            st, qn, bias_qeps, 1.0 / P, qr_t[:, h, :]))
                    else:
                        pending = (lambda st=st: finish(
                            st, kn, bias_keps, 1.0, KR[:, q0:q0 + SC]))
                    del st
                pending()
                # prefetch next chunk's x last so its Sync issues never sit
                # ahead of this chunk's latency-critical rot DMAs
                if c + 1 < NSC:
                    for i in range(16):
                        h2 = slice(i * 2, (i + 1) * 2)
                        nc.sync.dma_start(x_tiles[c + 1][:, h2, :],
                                          xT[:, h2, q0 + SC:q0 + 2 * SC])
                return qr_t

            def outproj_ops(c):
                """Generator of closures: output projection for chunk c,
                in PE-sized steps (one matmul per step). Used as filler
                between attention matmuls of chunk c+1."""
                q0 = c * SC
                wogs = {}

                def load_wog(ng):
                    wog = wogp.tile([P, G, SC], BF16, tag="wog")
                    for h in range(G):
                        nc.sync.dma_start(wog[:, h, :],
                                          woT[:, h, ng * SC:(ng + 1) * SC])
                    wogs[ng] = wog

                # first two groups' weights prefetched by the priming
                # next()-calls issued before proj(c+2) is emitted
                yield lambda: load_wog(0)
                yield lambda: load_wog(1)
                for ng in range(HID // SC):
                    wog = wogs[ng]
                    if ng + 2 < HID // SC:
                        yield lambda ng=ng: load_wog(ng + 2)
                    for stl in range(SC // P):
                        yp = psA.tile([P, SC], F32, tag="a",
                                      name=f"yp{c}_{ng}_{stl}")
                        for h in range(G):
                            yield lambda yp=yp, h=h, stl=stl, wog=wog, c=c: \
                                nc.tensor.matmul(
                                    yp, ots[(c, h)][:, stl * P:(stl + 1) * P],
                                    wog[:, h, :],
                                    start=(h == 0), stop=(h == G - 1))

                        def evac(yp=yp, stl=stl, ng=ng, q0=q0):
                            ys = ysp.tile([P, SC], BF16, tag="ys")
                            # vector only: a copy landing on ACT would evict
                            # the Exp table mid-attention (1.3us reload)
                            nc.vector.tensor_copy(ys, yp)
                            # store via the GpSimd SWDGE queue: keeps 128
                            # store issues off the Sync queue, and per-stl
                            # pieces land on parallel DMA engines
                            nc.gpsimd.dma_start(
                                y[q0 + stl * P:q0 + (stl + 1) * P,
                                  ng * SC:(ng + 1) * SC], ys)
                        yield evac

            def emit_attn(c, qr_t, filler):
                """Attention for chunk c; `filler` ops (outproj of c-1)
                are interleaved to keep PE busy while ACT runs exp."""
                def take(n):
                    for _ in range(n):
                        op = next(filler, None)
                        if op is None:
                            return
                        op()

                nkt = (c + 1) * 4
                for h in range(G):
                    avp = psD.tile([P, SC], F32, tag="d", name=f"av{c}_{h}")
                    dnp = psC.tile([P, SC], F32, tag="c", name=f"dn{c}_{h}")

                    def score_tile(kt, off):
                        ptps = psB.tile([P, SC], F32, tag="b",
                                        name=f"pt{c}_{h}_{kt}")
                        nc.tensor.matmul(
                            ptps[:, off:], KR[:, kt * P:(kt + 1) * P],
                            qr_t[:, h, off:], start=True, stop=True)
                        pt = ptp.tile([P, SC], BF16, tag="pt")
                        nc.scalar.activation(pt[:, off:], ptps[:, off:], Exp,
                                             bias=0.0, scale=1.0)
                        return pt

                    # full k-tiles, in pairs: one denominator matmul per
                    # pair (the pair-sum is a cheap bf16 DVE add)
                    for base in range(0, c * 4, 2):
                        pts = []
                        for kk in range(2):
                            kt = base + kk
                            pt = score_tile(kt, 0)
                            nc.tensor.matmul(avp, Vs[:, kt, :], pt,
                                             start=(kt == 0), stop=False)
                            pts.append(pt)
                            take(2)
                        psum2 = ptp.tile([P, SC], BF16, tag="pt")
                        nc.vector.tensor_add(psum2, pts[0], pts[1])
                        nc.tensor.matmul(dnp, ones_bf, psum2,
                                         start=(base == 0), stop=False)
                        take(1)
                    # diagonal k-tiles: causal-trimmed to valid q columns
                    for j in range(4):
                        kt = c * 4 + j
                        off = P * j
                        pt = score_tile(kt, off)
                        nc.vector.tensor_mul(pt[:, off:], pt[:, off:],
                                             masks[j][:, off:])
                        nc.tensor.matmul(dnp[:, off:], ones_bf, pt[:, off:],
                                         start=(kt == 0), stop=(j == 3))
                        nc.tensor.matmul(avp[:, off:], Vs[:, kt, :],
                                         pt[:, off:],
                                         start=(kt == 0), stop=(j == 3))
                        take(2)
                    rcp = rcpp.tile([P, SC], F32, tag="rcp")
                    nc.vector.reciprocal_approx_fast(rcp, dnp)
                    ot = otp.tile([P, SC], BF16, tag="ot", name=f"ot{c}_{h}")
                    nc.vector.tensor_mul(ot, avp, rcp)
                    ots[(c, h)] = ot
                    take(7)

            # ---- pipelined main loop ----
            qr_next = emit_proj(0)
            empty = iter(())
            for c in range(NSC):
                qr_cur = qr_next
                filler = outproj_ops(c - 1) if c >= 1 else empty
                if c + 1 < NSC:
                    qr_next = emit_proj(c + 1)
                emit_attn(c, qr_cur, filler)
                for op in filler:  # flush leftovers
                    op()
            for op in outproj_ops(NSC - 1):
                op()

    nc.finalize()
    return nc


def shard_inputs(x, wq, wk, wv, wo, q_norm_w, k_norm_w, cos_table, sin_table,
                 positions, **_ignored):
    """Host-side sharding: returns the list of 8 per-core input maps."""
    import ml_dtypes
    bf16 = ml_dtypes.bfloat16

    x = np.asarray(x, np.float32)
    pos = np.asarray(positions).astype(np.int64)
    cos_sel = np.asarray(cos_table, np.float32)[pos]   # [S, D]
    sin_sel = np.asarray(sin_table, np.float32)[pos]
    qw = np.ascontiguousarray(
        np.asarray(q_norm_w, np.float32).reshape(D, 1))
    kw = np.ascontiguousarray(
        np.asarray(k_norm_w, np.float32).reshape(D, 1))
    # fold rotate-half's minus sign into sin rows 0..63:
    # rope(z) = z*cos + [-z2; z1]*sin = z*cos + [z2; z1]*sin_eff
    sign = np.ones((1, D), np.float32)
    sign[0, :D // 2] = -1.0
    cost = np.ascontiguousarray(cos_sel.T).astype(bf16)            # [D, S]
    sint = np.ascontiguousarray((sin_sel * sign).T).astype(bf16)
    # x as [p, ht, s]: x[s, ht*128+p]
    xT3 = np.ascontiguousarray(
        x.reshape(S, NHT, P).transpose(2, 1, 0)).astype(bf16)
    wq = np.asarray(wq, np.float32)
    wk = np.asarray(wk, np.float32)
    wv = np.asarray(wv, np.float32)
    wo = np.asarray(wo, np.float32)

    in_maps = []
    for c in range(N_CORES):
        # weight shards, [p, ht, m] with p the contraction partition
        wq_s = wq[c * G * P:(c + 1) * G * P, :].T     # [HID, 512]
        wk_s = wk[c * P:(c + 1) * P, :].T             # [HID, 128]
        wv_s = wv[c * P:(c + 1) * P, :].T
        wo_s = wo[:, c * G * P:(c + 1) * G * P].T     # [512, HID]
        m = {
            "xT": xT3,
            "wqT": np.ascontiguousarray(
                wq_s.reshape(NHT, P, G * P).transpose(1, 0, 2)).astype(bf16),
            "wkT": np.ascontiguousarray(
                wk_s.reshape(NHT, P, P).transpose(1, 0, 2)).astype(bf16),
            "wvT": np.ascontiguousarray(
                wv_s.reshape(NHT, P, P).transpose(1, 0, 2)).astype(bf16),
            "woT": np.ascontiguousarray(
                wo_s.reshape(G, P, HID).transpose(1, 0, 2)).astype(bf16),
            "cost": cost, "sint": sint, "qnw": qw, "knw": kw,
        }
        in_maps.append(m)
    return in_maps


_NC = None


def _get_nc():
    global _NC
    if _NC is None:
        _NC = build_program()
    return _NC


def run_on_device(in_maps, trace=False):
    from concourse.bass_utils import run_bass_kernel_spmd
    nc = _get_nc()
    return run_bass_kernel_spmd(nc, in_maps, list(range(N_CORES)), trace=trace)


def kernel(**inputs):
    in_maps = shard_inputs(**inputs)
    res = run_on_device(in_maps).results
    y = np.zeros((S, HID), np.float32)
    for c in range(N_CORES):
        y += np.asarray(res[c]["y"], np.float32)
    return y.reshape(1, S, HID)
